# revision 22
# baseline (speedup 1.0000x reference)
"""Bass/Trainium2 kernel for nn_GATGCNNet (GAT conv -> GCN conv -> pool -> MLPs, two drug
branches + cell-line MLP fusion), distributed over 8 NeuronCores.

Sharding: cores 0-3 = drug branch 1, cores 4-7 = drug branch 2; within a branch each core
owns a contiguous 5120-node / 128-graph shard (dst-sharded edges).

Key structure (v2):
- GAT is factorized through its linearity: out_h[dst] = (sum_src alpha_h * x[src]) @ W_h,
  so the per-edge gather reads raw features x (77) + a_s (10) packed in 256B bf16 rows of a
  host-precomputed table that is replicated per core -> the first AllGather is eliminated.
- The GCN-phase node table (x1 rows) is bf16 and its AllGather is split into 4 row-quarters
  issued as phase B produces them, overlapping the collective with compute.
- All matmuls run in bf16 (f32 PSUM accumulation); biases are folded into the contraction
  as an extra all-ones row.
"""

import numpy as np
from ml_dtypes import bfloat16

import concourse.bass as bass
import concourse.bacc as bacc
import concourse.mybir as mybir
import concourse.tile as tile
from concourse import library_config
from concourse.bass_utils import run_bass_kernel_spmd

# problem constants (hardcoded per the task contract)
H, F, HF = 10, 77, 770
B, N, E = 512, 20480, 81920
NCORES = 8
R = 4                 # cores per branch
NSH = N // R          # nodes per core (5120)
NGRP = NSH // 128     # 128-node groups per core (40)
NPG = N // B          # nodes per graph (40)
GPC = B // R          # graphs per core-shard (128)
NQ = 8                # row-slices for the pipelined x1-table allgather
QGRP = NGRP // NQ     # groups per slice (5)
QROWS = 128 * QGRP    # rows per slice (640)
T1W = 128             # bf16 T1 row: [x 0:77 | a_s 77:87 | pad]  (256B)
T2W = 896             # bf16 T2 row: [x1 0:770 | pad]            (1792B)

f32 = mybir.dt.float32
bf16 = mybir.dt.bfloat16
i16 = mybir.dt.int16


def _wrap_idx(a):
    """[n] int array (n % 16 == 0) -> [128, n/16] int16 wrapped layout, replicated x8."""
    w = a.astype(np.int16).reshape(-1, 16).T
    return np.tile(w, (8, 1))


def _pack_k(w, nkt, m, dtype=bfloat16):
    """Pad [K, M] -> [128*nkt, M] with zeros, return [128, nkt, M] (row 128*kt+p at [p, kt])."""
    w = np.asarray(w, np.float32)
    kp = np.zeros((128 * nkt, m), np.float32)
    kp[: w.shape[0]] = w
    return np.ascontiguousarray(kp.reshape(nkt, 128, m).transpose(1, 0, 2)).astype(dtype)


def _bias_cols(b, nm):
    """bias [M] -> [128, nm] f32 with tile m's bias in column m (padded)."""
    bp = np.zeros(128 * nm, np.float32)
    bp[: len(b)] = b
    return np.ascontiguousarray(bp.reshape(nm, 128).T)


def _fcg1_pack(w):
    """fcg1_W [1540, 1500] -> [128, 14, 1536] bf16: kt0-6 = mx rows, kt7-13 = mean rows."""
    out = np.zeros((14, 128, 1536), np.float32)
    for j, base in ((0, 0), (7, HF)):
        for t in range(7):
            lo = base + 128 * t
            hi = min(base + HF, lo + 128)
            if hi > lo:
                out[j + t, : hi - lo, :1500] = w[lo:hi]
    return np.ascontiguousarray(out.transpose(1, 0, 2)).astype(bfloat16)


def _t2_row_of(n):
    """Global node id -> row in the quarter-blocked T2_full layout."""
    c, l = n // NSH, n % NSH
    q = l // QROWS
    return R * QROWS * q + QROWS * c + (l % QROWS)


def _edge_structure(edge_lists):
    """Max per-group chunk count across all (branch, core) entries (shared => SPMD)."""
    cnts = np.zeros((len(edge_lists), NGRP), np.int64)
    for i, (srcs, dstl) in enumerate(edge_lists):
        cnts[i] = np.bincount(dstl // 128, minlength=NGRP)
    C = np.maximum(1, np.ceil(cnts.max(axis=0) / 128).astype(np.int64))
    return C


def _build_host_inputs(inputs):
    per_branch = {}
    edge_lists = []
    for d in ("1", "2"):
        ei = np.asarray(inputs["edge_index" + d])
        src = np.concatenate([ei[0], np.arange(N, dtype=ei.dtype)]).astype(np.int64)
        dst = np.concatenate([ei[1], np.arange(N, dtype=ei.dtype)]).astype(np.int64)
        order = np.argsort(dst, kind="stable")
        src, dst = src[order], dst[order]
        deg = np.bincount(dst, minlength=N).astype(np.float32)
        dinv = 1.0 / np.sqrt(np.maximum(deg, 1.0))
        batch = np.asarray(inputs["batch_d" + d])
        assert np.array_equal(batch, np.repeat(np.arange(B, dtype=batch.dtype), NPG)), \
            "kernel assumes uniform contiguous 40-node graphs"

        # host precompute: attention projections a_s/a_d for all nodes (O(N*20))
        x = np.asarray(inputs["xd" + d], np.float32)
        gat_W = np.asarray(inputs["gat_W" + d], np.float32)            # [77, 770]
        att_s = np.asarray(inputs["gat_as" + d], np.float32)            # [10, 77]
        att_d = np.asarray(inputs["gat_ad" + d], np.float32)
        WA = np.zeros((77, 2 * H), np.float32)
        for h in range(H):
            Wh = gat_W[:, h * F:(h + 1) * F]
            WA[:, h] = Wh @ att_s[h]
            WA[:, H + h] = Wh @ att_d[h]
        a_sd = x @ WA                                                   # [N, 20]

        # T1 table: [x | 1 | pad] bf16, 256B rows, replicated per branch core
        T1 = np.zeros((N, T1W), np.float32)
        T1[:, 0:77] = x
        T1[:, 77] = 1.0
        per_branch[d] = dict(src=src, dst=dst, dinv=dinv,
                             T1=np.ascontiguousarray(T1).astype(bfloat16),
                             a_s=a_sd[:, 0:H], a_d=a_sd[:, H:2 * H])
        for r in range(R):
            lo, hi = NSH * r, NSH * (r + 1)
            m = (dst >= lo) & (dst < hi)
            edge_lists.append((src[m], dst[m] - lo))

    C = _edge_structure(edge_lists)
    NCHUNK = int(C.sum())
    TOTCOL = 8 * NCHUNK
    group_cols = np.zeros(NGRP, np.int64)
    acc = 0
    for g in range(NGRP):
        group_cols[g] = acc
        acc += 8 * int(C[g])
    amax = max(float(np.max(per_branch[d]["a_s"]) + np.max(per_branch[d]["a_d"]))
               for d in ("1", "2"))
    struct = dict(C=[int(c) for c in C], group_cols=[int(c) for c in group_cols],
                  NCHUNK=NCHUNK, TOTCOL=TOTCOL, need_clamp=bool(amax >= 80.0))

    t2row = _t2_row_of(np.arange(N))

    core_edge = {}
    for ci in range(NCORES):
        d = "1" if ci < R else "2"
        r = ci % R
        srcs, dstl = edge_lists[(0 if ci < R else R) + r]
        gids = dstl // 128
        idx_b = np.zeros((128, TOTCOL), np.int16)
        idx_c = np.zeros((128, TOTCOL), np.int16)
        ld_col = np.full((128, NCHUNK), 255.0, np.float32)
        adc = np.zeros((128, NCHUNK, H), np.float32)   # a_s[src_e]+a_d[dst_e] per edge
        a_s = per_branch[d]["a_s"]
        a_d = per_branch[d]["a_d"]
        kbase = 0
        for g in range(NGRP):
            m = gids == g
            gs = srcs[m]
            gd = dstl[m] + NSH * r                      # global dst id
            gl = dstl[m] - 128 * g
            cap = 128 * int(C[g])
            padn = cap - len(gs)
            gs_p = np.concatenate([gs, np.zeros(padn, np.int64)])
            gd_p = np.concatenate([gd, np.zeros(padn, np.int64)])
            gl_p = np.concatenate([gl, np.full(padn, 255, np.int64)])
            cc = int(group_cols[g])
            idx_b[:, cc:cc + 8 * int(C[g])] = _wrap_idx(gs_p)
            idx_c[:, cc:cc + 8 * int(C[g])] = _wrap_idx(t2row[gs_p])
            for k in range(int(C[g])):
                ld_col[:, kbase + k] = gl_p[k * 128:(k + 1) * 128]
                adc[:, kbase + k, :] = (a_s[gs_p[k * 128:(k + 1) * 128]]
                                        + a_d[gd_p[k * 128:(k + 1) * 128]])
            kbase += int(C[g])
        pb = per_branch[d]
        dinv_sh = pb["dinv"][NSH * r:NSH * (r + 1)]
        core_edge[ci] = dict(
            idx_b=idx_b, idx_c=idx_c, ld=ld_col,
            dinv=np.ascontiguousarray(dinv_sh.reshape(NGRP, 128).T),
            adc=adc.astype(bfloat16),
        )

    wmaps = {}
    for d in ("1", "2"):
        gat_W = np.asarray(inputs["gat_W" + d], np.float32)
        Wp77 = np.zeros((128, H, F), np.float32)                       # per-head blocks
        for h in range(H):
            Wp77[0:77, h, :] = gat_W[:, h * F:(h + 1) * F]
        wmaps[d] = dict(
            Wp77=Wp77.astype(bfloat16),                                # [128,10,77] bf16
            gcn_W_p=_pack_k(np.asarray(inputs["gcn_W" + d], np.float32), 7, HF),
            gat_brow=np.asarray(inputs["gat_b" + d], np.float32)
                .reshape(1, HF).astype(bfloat16),
            gcn_brow=np.asarray(inputs["gcn_b" + d], np.float32)
                .reshape(1, HF).astype(bfloat16),
            fcg1_W_p=_fcg1_pack(np.asarray(inputs["fcg1_W" + d], np.float32)),
            fcg1_b=_bias_cols(np.asarray(inputs["fcg1_b" + d], np.float32), 12),
        )

    fcg2_W_p = _pack_k(np.asarray(inputs["fcg2_W"], np.float32), 12, 128)
    fcg2_b = _bias_cols(np.asarray(inputs["fcg2_b"], np.float32), 1)
    xcT = np.concatenate([inputs["xc1"], inputs["xc2"], inputs["xc3"], inputs["xtc"]],
                         axis=1).astype(np.float32).T                   # [1546, 512]
    xcT_p = _pack_k(xcT, 13, B)
    cl_W1 = np.asarray(inputs["cl_W1"], np.float32)
    cl_b1 = np.asarray(inputs["cl_b1"], np.float32)
    cl_W2 = np.asarray(inputs["cl_W2"], np.float32)
    cl_b2 = _bias_cols(np.asarray(inputs["cl_b2"], np.float32), 2)
    fc1_W = np.asarray(inputs["fc1_W"], np.float32)
    fc1_b = np.asarray(inputs["fc1_b"], np.float32)
    fc2_W = np.asarray(inputs["fc2_W"], np.float32)
    fc2_b = _bias_cols(np.asarray(inputs["fc2_b"], np.float32), 2)
    out_W_p = _pack_k(np.asarray(inputs["out_W"], np.float32), 3, 1)
    out_b = np.asarray(inputs["out_b"], np.float32).reshape(1, 1)

    iota_row = np.tile(np.arange(128, dtype=np.float32)[None, :], (128, 1)).astype(bfloat16)
    ident = np.eye(128, dtype=np.float32).astype(bfloat16)

    in_maps = []
    for ci in range(NCORES):
        d = "1" if ci < R else "2"
        w = wmaps[d]
        ce = core_edge[ci]
        in_maps.append(dict(
            T1=per_branch[d]["T1"],
            Wp77=w["Wp77"], gcn_W_p=w["gcn_W_p"],
            gat_brow=w["gat_brow"], gcn_brow=w["gcn_brow"],
            fcg1_W_p=w["fcg1_W_p"], fcg1_b=w["fcg1_b"],
            fcg2_W_p=fcg2_W_p, fcg2_b=fcg2_b,
            xcT_p=xcT_p, cl_b2=cl_b2, fc2_b=fc2_b,
            cl_W1s=_pack_k(cl_W1[:, 128 * ci:128 * (ci + 1)], 13, 128),
            cl_b1s=_bias_cols(cl_b1[128 * ci:128 * (ci + 1)], 1),
            cl_W2s=cl_W2[128 * ci:128 * (ci + 1), :].astype(bfloat16),
            fc1_Ws=_pack_k(fc1_W[:, 256 * ci:256 * (ci + 1)], 5, 256),
            fc1_bs=_bias_cols(fc1_b[256 * ci:256 * (ci + 1)], 2),
            fc2_Ws=_pack_k(fc2_W[256 * ci:256 * (ci + 1), :], 2, 256),
            out_W_p=out_W_p, out_b=out_b,
            idx_b=ce["idx_b"], idx_c=ce["idx_c"], ld=ce["ld"], dinv=ce["dinv"],
            adc=ce["adc"], iota=iota_row, ident=ident,
        ))
    return in_maps, struct


def _build_program(struct):
    C = struct["C"]
    group_cols = struct["group_cols"]
    NCHUNK = struct["NCHUNK"]
    TOTCOL = struct["TOTCOL"]
    need_clamp = struct.get("need_clamp", True)
    AluOp = mybir.AluOpType
    Act = mybir.ActivationFunctionType

    nc = bacc.Bacc("TRN2", target_bir_lowering=False, debug=False, num_devices=NCORES)

    # --- inputs ---
    T1_in = nc.dram_tensor("T1", [N, T1W], bf16, kind="ExternalInput")
    Wp77_in = nc.dram_tensor("Wp77", [128, H, F], bf16, kind="ExternalInput")
    gcn_W_in = nc.dram_tensor("gcn_W_p", [128, 7, HF], bf16, kind="ExternalInput")
    gat_brow_in = nc.dram_tensor("gat_brow", [1, HF], bf16, kind="ExternalInput")
    gcn_brow_in = nc.dram_tensor("gcn_brow", [1, HF], bf16, kind="ExternalInput")
    fcg1_W_in = nc.dram_tensor("fcg1_W_p", [128, 14, 1536], bf16, kind="ExternalInput")
    fcg1_b_in = nc.dram_tensor("fcg1_b", [128, 12], f32, kind="ExternalInput")
    fcg2_W_in = nc.dram_tensor("fcg2_W_p", [128, 12, 128], bf16, kind="ExternalInput")
    fcg2_b_in = nc.dram_tensor("fcg2_b", [128, 1], f32, kind="ExternalInput")
    xcT_in = nc.dram_tensor("xcT_p", [128, 13, B], bf16, kind="ExternalInput")
    cl_W1s_in = nc.dram_tensor("cl_W1s", [128, 13, 128], bf16, kind="ExternalInput")
    cl_b1s_in = nc.dram_tensor("cl_b1s", [128, 1], f32, kind="ExternalInput")
    cl_W2s_in = nc.dram_tensor("cl_W2s", [128, 256], bf16, kind="ExternalInput")
    cl_b2_in = nc.dram_tensor("cl_b2", [128, 2], f32, kind="ExternalInput")
    fc1_Ws_in = nc.dram_tensor("fc1_Ws", [128, 5, 256], bf16, kind="ExternalInput")
    fc1_bs_in = nc.dram_tensor("fc1_bs", [128, 2], f32, kind="ExternalInput")
    fc2_Ws_in = nc.dram_tensor("fc2_Ws", [128, 2, 256], bf16, kind="ExternalInput")
    fc2_b_in = nc.dram_tensor("fc2_b", [128, 2], f32, kind="ExternalInput")
    out_W_in = nc.dram_tensor("out_W_p", [128, 3, 1], bf16, kind="ExternalInput")
    out_b_in = nc.dram_tensor("out_b", [1, 1], f32, kind="ExternalInput")
    idx_b_in = nc.dram_tensor("idx_b", [128, TOTCOL], i16, kind="ExternalInput")
    idx_c_in = nc.dram_tensor("idx_c", [128, TOTCOL], i16, kind="ExternalInput")
    ld_in = nc.dram_tensor("ld", [128, NCHUNK], f32, kind="ExternalInput")
    dinv_in = nc.dram_tensor("dinv", [128, NGRP], f32, kind="ExternalInput")
    adc_in = nc.dram_tensor("adc", [128, NCHUNK, H], bf16, kind="ExternalInput")
    iota_in = nc.dram_tensor("iota", [128, 128], bf16, kind="ExternalInput")
    ident_in = nc.dram_tensor("ident", [128, 128], bf16, kind="ExternalInput")

    out_t = nc.dram_tensor("out", [B, 1], f32, kind="ExternalOutput")

    # --- internal DRAM ---
    T2_loc_q = [nc.dram_tensor(f"T2_loc_q{q}", [QROWS, T2W], bf16) for q in range(NQ)]
    T2_full = nc.dram_tensor("T2_full", [N, T2W], bf16)
    x2T_dram = nc.dram_tensor("x2T_dram", [7 * 128, NSH], bf16)
    Hg_loc = nc.dram_tensor("Hg_loc", [128, GPC], bf16)
    H_full = nc.dram_tensor("H_full", [128 * NCORES, GPC], bf16, addr_space="Shared")
    xcl_part = nc.dram_tensor("xcl_part", [128, 2, B], f32)
    xcl_sum = nc.dram_tensor("xcl_sum", [128, 2, B], f32)
    z4_part = nc.dram_tensor("z4_part", [128, 2, B], f32)
    z4_sum = nc.dram_tensor("z4_sum", [128, 2, B], f32)

    groups_b1 = [[0, 1, 2, 3], [4, 5, 6, 7]]
    groups_all = [list(range(NCORES))]

    with tile.TileContext(nc) as tc:
        nc.gpsimd.load_library(library_config.mlp)
        with tc.tile_pool(name="const", bufs=1) as cpool:
            iota = cpool.tile([128, 128], bf16)
            nc.sync.dma_start(iota[:], iota_in[:])
            ident = cpool.tile([128, 128], bf16)
            nc.sync.dma_start(ident[:], ident_in[:])
            idx_b = cpool.tile([128, TOTCOL], i16)
            nc.sync.dma_start(idx_b[:], idx_b_in[:])
            idx_c = cpool.tile([128, TOTCOL], i16)
            nc.sync.dma_start(idx_c[:], idx_c_in[:])
            ldc = cpool.tile([128, NCHUNK], f32)
            nc.sync.dma_start(ldc[:], ld_in[:])
            dinv = cpool.tile([128, NGRP], f32)
            nc.sync.dma_start(dinv[:], dinv_in[:])
            adc = cpool.tile([128, NCHUNK, H], bf16)
            nc.sync.dma_start(adc[:, :, :], adc_in[:, :, :])
            Wp77 = cpool.tile([128, H, F], bf16)
            nc.sync.dma_start(Wp77[:, :, :], Wp77_in[:, :, :])
            gat_brow = cpool.tile([1, HF], bf16)
            nc.sync.dma_start(gat_brow[:], gat_brow_in[:])
            gcn_brow = cpool.tile([1, HF], bf16)
            nc.sync.dma_start(gcn_brow[:], gcn_brow_in[:])
            ones1 = cpool.tile([1, 128], bf16)
            nc.vector.memset(ones1[:], 1.0)
            xcT = cpool.tile([128, 13, B], bf16)
            nc.sync.dma_start(xcT[:, :, :], xcT_in[:, :, :])

            # -------- cell-line MLP, feature-sharded; partial sum AllReduced --------
            # (runs early on the tensor engine; the AllReduce overlaps phase B)
            with tc.tile_pool(name="phCL", bufs=1) as clp, \
                 tc.tile_pool(name="psCL", bufs=1, space="PSUM") as psCL:
                cl_W1s = clp.tile([128, 13, 128], bf16)
                nc.sync.dma_start(cl_W1s[:, :, :], cl_W1s_in[:, :, :])
                cl_b1s = clp.tile([128, 1], f32)
                nc.sync.dma_start(cl_b1s[:], cl_b1s_in[:])
                cl_W2s = clp.tile([128, 256], bf16)
                nc.sync.dma_start(cl_W2s[:], cl_W2s_in[:])
                psz = psCL.tile([128, B], f32, tag="z")
                for kt in range(13):
                    nc.tensor.matmul(psz[:], cl_W1s[:, kt, :], xcT[:, kt, :],
                                     start=(kt == 0), stop=(kt == 12))
                zc1 = clp.tile([128, B], bf16)
                nc.scalar.activation(zc1[:], psz[:], Act.Relu, bias=cl_b1s[:, 0:1])
                xclp = clp.tile([128, 2, B], f32)
                for mt in range(2):
                    psz2 = psCL.tile([128, B], f32, tag="z2")
                    nc.tensor.matmul(psz2[:], cl_W2s[:, 128 * mt:128 * (mt + 1)],
                                     zc1[:], start=True, stop=True)
                    nc.scalar.activation(xclp[:, mt, :], psz2[:], Act.Copy)
                nc.sync.dma_start(xcl_part[:, :, :], xclp[:, :, :])
            nc.gpsimd.collective_compute(
                "AllReduce", mybir.AluOpType.add, replica_groups=groups_all,
                ins=[xcl_part[:, :, :]], outs=[xcl_sum[:, :, :]])

            # ---------------- Phase B: GAT conv (factorized) ----------------
            with tc.tile_pool(name="phB", bufs=3) as bpool, \
                 tc.tile_pool(name="phBsk", bufs=3) as skpool, \
                 tc.tile_pool(name="phBs", bufs=3) as spool, \
                 tc.tile_pool(name="phBt", bufs=2) as tpool, \
                 tc.tile_pool(name="psB", bufs=2, space="PSUM") as psB, \
                 tc.tile_pool(name="psBt", bufs=1, space="PSUM") as psT, \
                 tc.tile_pool(name="psBx", bufs=1, space="PSUM") as psX:
                kbase = 0
                for g in range(NGRP):
                    Cg = C[g]
                    cc = group_cols[g]
                    G = bpool.tile([128, Cg, T1W], bf16, tag="G")
                    nc.gpsimd.dma_gather(
                        G[:, :, :], T1_in[:, :], idx_b[:, cc:cc + 8 * Cg],
                        128 * Cg, 128 * Cg, T1W)
                    # alpha -> e for all chunks of the group at once
                    al = spool.tile([128, Cg, H], f32, tag="al")
                    asl = adc[:, kbase:kbase + Cg, :]
                    nc.vector.scalar_tensor_tensor(al[:, :, :], asl, 0.2,
                                                   asl, AluOp.mult, AluOp.max)
                    if need_clamp:
                        nc.vector.tensor_scalar(al[:, :, :], al[:, :, :], 60.0, None,
                                                AluOp.min)
                    ee = spool.tile([128, Cg, H], bf16, tag="ee")
                    nc.scalar.activation(ee[:, :, :], al[:, :, :], Act.Exp)
                    # select matrices for all chunks in one op
                    Sk_all = skpool.tile([128, Cg, 128], bf16, tag="Sk")
                    nc.vector.tensor_tensor(
                        Sk_all[:, :, :],
                        ldc[:, kbase:kbase + Cg].rearrange("p (c o) -> p c o", o=1)
                            .broadcast_to((128, Cg, 128)),
                        iota[:, :].rearrange("p (o j) -> p o j", o=1)
                            .broadcast_to((128, Cg, 128)),
                        AluOp.is_equal)
                    # weighted messages [e*x | e] per head (78 cols each); heads are
                    # split between the vector and gpsimd engines to balance load
                    Gw = spool.tile([128, Cg, HF + H], bf16, tag="Gw")
                    HS = 5
                    for eng, h0, h1 in ((nc.vector, 0, HS), (nc.gpsimd, HS, H)):
                        e4 = ee[:, :, h0:h1].rearrange("p c (h o) -> p c h o", o=1) \
                            .broadcast_to((128, Cg, h1 - h0, F + 1))
                        x4 = G[:, :, 0:78].rearrange("p c (o f) -> p c o f", o=1) \
                            .broadcast_to((128, Cg, h1 - h0, F + 1))
                        eng.tensor_tensor(
                            Gw[:, :, (F + 1) * h0:(F + 1) * h1]
                                .rearrange("p c (h f) -> p c h f", f=F + 1),
                            e4, x4, AluOp.mult)
                    pg = psB.tile([128, HF + H], f32, tag="agg")
                    for k in range(Cg):
                        nc.tensor.matmul(pg[:, 0:512], Sk_all[:, k, :], Gw[:, k, 0:512],
                                         start=(k == 0), stop=(k == Cg - 1))
                        nc.tensor.matmul(pg[:, 512:HF + H], Sk_all[:, k, :],
                                         Gw[:, k, 512:HF + H],
                                         start=(k == 0), stop=(k == Cg - 1))
                    kbase += Cg
                    # normalize, transpose, apply block-diag W (+bias row), ELU, *dinv
                    pgs = spool.tile([128, HF + H], f32, tag="pgs")
                    nc.scalar.activation(pgs[:], pg[:], Act.Copy)
                    pg4 = pgs[:].rearrange("p (h f) -> p h f", f=F + 1)
                    rden = spool.tile([128, H], f32, tag="rden")
                    nc.vector.reciprocal(rden[:], pg4[:, :, F:F + 1]
                                         .rearrange("p h o -> p (h o)"))
                    aggn = spool.tile([128, HF], bf16, tag="aggn")
                    r3 = rden[:].rearrange("p (h o) -> p h o", o=1) \
                        .broadcast_to((128, H, F))
                    nc.vector.tensor_tensor(
                        aggn[:].rearrange("p (h f) -> p h f", f=F),
                        pg4[:, :, 0:F], r3, AluOp.mult)
                    aggnT = tpool.tile([128, H, 128], bf16, tag="aT")
                    ptb = psT.tile([128, H, 128], bf16, tag="tr")
                    for h in range(H):
                        nc.tensor.transpose(ptb[0:77, h, :],
                                            aggn[:, F * h:F * (h + 1)], ident[:])
                    nc.scalar.activation(aggnT[:, :, :], ptb[:, :, :], Act.Copy)
                    # W output: heads 0-5 at cols 77h (bank 0), heads 6-9 at
                    # 512+77(h-6) (bank 1) -- no matmul crosses a PSUM bank.
                    # Bias is accumulated via a K=1 all-ones matmul per head.
                    ps_x1 = psX.tile([128, 1024], f32, tag="x1")
                    for h in range(H):
                        off = 77 * h if h < 6 else 512 + 77 * (h - 6)
                        nc.tensor.matmul(ps_x1[:, off:off + F], aggnT[0:77, h, :],
                                         Wp77[0:77, h, :], start=True, stop=False)
                        nc.tensor.matmul(ps_x1[:, off:off + F], ones1[0:1, :],
                                         gat_brow[0:1, F * h:F * (h + 1)],
                                         start=False, stop=True)
                    m = spool.tile([128, 1024], bf16, tag="m")
                    nc.vector.tensor_scalar(m[:], ps_x1[:], 0.0, None, AluOp.min)
                    e2 = spool.tile([128, 1024], bf16, tag="e2")
                    nc.scalar.activation(e2[:], m[:], Act.Exp)
                    x1f = spool.tile([128, 1024], f32, tag="x1f")
                    nc.vector.scalar_tensor_tensor(x1f[:], e2[:], -1.0, ps_x1[:],
                                                   AluOp.add, AluOp.max)
                    T2row = bpool.tile([128, T2W], bf16, tag="T2row")
                    nc.scalar.activation(T2row[:, 0:462], x1f[:, 0:462], Act.Copy,
                                         scale=dinv[:, g:g + 1])
                    nc.scalar.activation(T2row[:, 462:HF], x1f[:, 512:820], Act.Copy,
                                         scale=dinv[:, g:g + 1])
                    q, j = g // QGRP, g % QGRP
                    nc.sync.dma_start(T2_loc_q[q][128 * j:128 * (j + 1), 0:HF], T2row[:, 0:HF])
                    if j == QGRP - 1:
                        nc.gpsimd.collective_compute(
                            "AllGather", mybir.AluOpType.bypass,
                            replica_groups=groups_b1,
                            ins=[T2_loc_q[q][:, :]],
                            outs=[T2_full[R * QROWS * q:R * QROWS * (q + 1), :]])

            gcn_W = cpool.tile([128, 7, HF], bf16)
            nc.sync.dma_start(gcn_W[:, :, :], gcn_W_in[:, :, :])

            # ---------------- Phase C: GCN conv ----------------
            # Aggregate per 128-dst group; batch 4 groups into a 512-node super-group and
            # produce x2T directly: x2T[fo, n] = relu(sum_fi W[fi, fo] * aggT[fi, n] + b).
            with tc.tile_pool(name="phC", bufs=2) as cpool2, \
                 tc.tile_pool(name="phCsk", bufs=2) as skpool2, \
                 tc.tile_pool(name="phCs", bufs=2) as cspool, \
                 tc.tile_pool(name="phCt", bufs=2) as tpool2, \
                 tc.tile_pool(name="psC", bufs=2, space="PSUM") as psC, \
                 tc.tile_pool(name="psCt", bufs=2, space="PSUM") as psCt, \
                 tc.tile_pool(name="psCx", bufs=2, space="PSUM") as psCx:
                onesd = cpool.tile([1, 512], bf16)
                nc.vector.memset(onesd[:], 1.0)
                kbase = 0
                for sg in range(NGRP // 4):
                    aggT_sg = tpool2.tile([128, 7, 512], bf16, tag="aggT")
                    for gi in range(4):
                        g = 4 * sg + gi
                        Cg = C[g]
                        cc = group_cols[g]
                        G2 = cpool2.tile([128, Cg, T2W], bf16, tag="G2")
                        nc.gpsimd.dma_gather(
                            G2[:, :, :], T2_full[:, :], idx_c[:, cc:cc + 8 * Cg],
                            128 * Cg, 128 * Cg, T2W)
                        Sk_all = skpool2.tile([128, Cg, 128], bf16, tag="S2")
                        nc.vector.tensor_tensor(
                            Sk_all[:, :, :],
                            ldc[:, kbase:kbase + Cg].rearrange("p (c o) -> p c o", o=1)
                                .broadcast_to((128, Cg, 128)),
                            iota[:, :].rearrange("p (o j) -> p o j", o=1)
                                .broadcast_to((128, Cg, 128)),
                            AluOp.is_equal)
                        pg = psC.tile([128, HF], f32, tag="acc2")
                        for k in range(Cg):
                            nc.tensor.matmul(pg[:, 0:512], Sk_all[:, k, :],
                                             G2[:, k, 0:512],
                                             start=(k == 0), stop=(k == Cg - 1))
                            nc.tensor.matmul(pg[:, 512:HF], Sk_all[:, k, :],
                                             G2[:, k, 512:HF],
                                             start=(k == 0), stop=(k == Cg - 1))
                            kbase += 1
                        agg = cspool.tile([128, HF], bf16, tag="agg")
                        nc.scalar.activation(agg[:], pg[:], Act.Copy,
                                             scale=dinv[:, g:g + 1])
                        ptb = psCt.tile([128, 7, 128], bf16, tag="tr")
                        for ft in range(7):
                            kf = min(128, HF - 128 * ft)
                            nc.tensor.transpose(ptb[0:kf, ft, :],
                                                agg[:, 128 * ft:128 * ft + kf], ident[:])
                        nc.scalar.activation(aggT_sg[:, :, 128 * gi:128 * (gi + 1)],
                                             ptb[:, :, :], Act.Copy)
                    x2T_sb = cspool.tile([128, 7, 512], bf16, tag="x2T")
                    for mt in range(7):
                        mm = min(128, HF - 128 * mt)
                        psx = psCx.tile([128, 512], f32, tag="x2ps")
                        for kt in range(7):
                            kf = min(128, HF - 128 * kt)
                            nc.tensor.matmul(psx[0:mm, :],
                                             gcn_W[0:kf, kt, 128 * mt:128 * mt + mm],
                                             aggT_sg[0:kf, kt, :],
                                             start=(kt == 0), stop=False)
                        nc.tensor.matmul(psx[0:mm, :],
                                         gcn_brow[0:1, 128 * mt:128 * mt + mm],
                                         onesd[0:1, :], start=False, stop=True)
                        nc.scalar.activation(x2T_sb[0:mm, mt, :], psx[0:mm, :], Act.Relu)
                        nc.sync.dma_start(
                            x2T_dram[128 * mt:128 * mt + mm, 512 * sg:512 * (sg + 1)],
                            x2T_sb[0:mm, mt, :])

            # ---------------- Phase D: pooling ----------------
            with tc.tile_pool(name="pooled", bufs=1) as plpool:
                pooled = plpool.tile([128, 14, 128], bf16)
                nc.vector.memset(pooled[:, 6, :], 0.0)
                nc.vector.memset(pooled[:, 13, :], 0.0)
                with tc.tile_pool(name="phD", bufs=2) as dpool:
                    for ft in range(7):
                        kf = min(128, HF - 128 * ft)
                        strip = dpool.tile([128, NSH], bf16, tag="strip")
                        nc.sync.dma_start(strip[0:kf, :],
                                          x2T_dram[128 * ft:128 * ft + kf, :])
                        seg = strip[0:kf, :].rearrange("p (gr n) -> p gr n", n=NPG)
                        nc.vector.tensor_reduce(pooled[0:kf, ft, :], seg,
                                                mybir.AxisListType.X, AluOp.max)
                        sm = dpool.tile([128, 128], f32, tag="sm")
                        nc.vector.tensor_reduce(sm[0:kf, :], seg,
                                                mybir.AxisListType.X, AluOp.add)
                        nc.vector.tensor_scalar(pooled[0:kf, 7 + ft, :], sm[0:kf, :],
                                                1.0 / NPG, None, AluOp.mult)

                # ---------------- Phase E: fcg1 / fcg2 ----------------
                with tc.tile_pool(name="phE", bufs=1) as epool, \
                     tc.tile_pool(name="psE", bufs=2, space="PSUM") as psE:
                    fcg1_W = epool.tile([128, 14, 1536], bf16)
                    nc.sync.dma_start(fcg1_W[:, :, :], fcg1_W_in[:, :, :])
                    fcg1_b = epool.tile([128, 12], f32)
                    nc.sync.dma_start(fcg1_b[:], fcg1_b_in[:])
                    fcg2_W = epool.tile([128, 12, 128], bf16)
                    nc.sync.dma_start(fcg2_W[:, :, :], fcg2_W_in[:, :, :])
                    fcg2_b = epool.tile([128, 1], f32)
                    nc.sync.dma_start(fcg2_b[:], fcg2_b_in[:])
                    z1T = epool.tile([128, 12, 128], bf16)
                    for mt in range(12):
                        psz = psE.tile([128, 128], f32, tag="ze")
                        for kt in range(14):
                            nc.tensor.matmul(psz[:], fcg1_W[:, kt, 128 * mt:128 * (mt + 1)],
                                             pooled[:, kt, :], start=(kt == 0), stop=(kt == 13))
                        nc.scalar.activation(z1T[:, mt, :], psz[:], Act.Relu,
                                             bias=fcg1_b[:, mt:mt + 1])
                    psh = psE.tile([128, GPC], f32, tag="he")
                    for kt in range(12):
                        nc.tensor.matmul(psh[:], fcg2_W[:, kt, :], z1T[:, kt, :],
                                         start=(kt == 0), stop=(kt == 11))
                    hdT = epool.tile([128, GPC], bf16)
                    nc.scalar.activation(hdT[:], psh[:], Act.Relu, bias=fcg2_b[:, 0:1])
                    nc.sync.dma_start(Hg_loc[:, :], hdT[:])

            nc.gpsimd.collective_compute(
                "AllGather", mybir.AluOpType.bypass, replica_groups=groups_all,
                ins=[Hg_loc[:, :]], outs=[H_full[:, :]])

            # ---------------- Phase F: fusion MLP (feature-sharded) ----------------
            with tc.tile_pool(name="phFa", bufs=1) as fpool, \
                 tc.tile_pool(name="psF", bufs=2, space="PSUM") as psF:
                cl_b2 = fpool.tile([128, 2], f32)
                nc.sync.dma_start(cl_b2[:], cl_b2_in[:])
                xcls = fpool.tile([128, 2, B], f32)
                nc.sync.dma_start(xcls[:, :, :], xcl_sum[:, :, :])
                xclT = fpool.tile([128, 2, B], bf16)
                for mt in range(2):
                    nc.scalar.activation(xclT[:, mt, :], xcls[:, mt, :], Act.Relu,
                                         bias=cl_b2[:, mt:mt + 1])
                h1T = fpool.tile([128, B], bf16)
                h2T = fpool.tile([128, B], bf16)
                for r in range(R):
                    nc.sync.dma_start(h1T[:, GPC * r:GPC * (r + 1)],
                                      H_full[128 * r:128 * (r + 1), :])
                    nc.sync.dma_start(h2T[:, GPC * r:GPC * (r + 1)],
                                      H_full[128 * (R + r):128 * (R + r + 1), :])
                xtcT = xcT[0:10, 12, :]   # xtc rows live at packed rows 1536:1546
                fc1_Ws = fpool.tile([128, 5, 256], bf16)
                nc.sync.dma_start(fc1_Ws[:, :, :], fc1_Ws_in[:, :, :])
                fc1_bs = fpool.tile([128, 2], f32)
                nc.sync.dma_start(fc1_bs[:], fc1_bs_in[:])
                rhs1 = [h1T[:, :], h2T[:, :], xclT[:, 0, :], xclT[:, 1, :], xtcT]
                z3T = fpool.tile([128, 2, B], bf16)
                for mt in range(2):
                    psz = psF.tile([128, B], f32, tag="zf")
                    for kt in range(5):
                        kf = 10 if kt == 4 else 128
                        nc.tensor.matmul(psz[:], fc1_Ws[0:kf, kt, 128 * mt:128 * (mt + 1)],
                                         rhs1[kt], start=(kt == 0), stop=(kt == 4))
                    nc.scalar.activation(z3T[:, mt, :], psz[:], Act.Relu,
                                         bias=fc1_bs[:, mt:mt + 1])
                fc2_Ws = fpool.tile([128, 2, 256], bf16)
                nc.sync.dma_start(fc2_Ws[:, :, :], fc2_Ws_in[:, :, :])
                z4p = fpool.tile([128, 2, B], f32)
                for mt in range(2):
                    psz = psF.tile([128, B], f32, tag="zf")
                    for kt in range(2):
                        nc.tensor.matmul(psz[:], fc2_Ws[:, kt, 128 * mt:128 * (mt + 1)],
                                         z3T[:, kt, :], start=(kt == 0), stop=(kt == 1))
                    nc.scalar.activation(z4p[:, mt, :], psz[:], Act.Copy)
                nc.sync.dma_start(z4_part[:, :, :], z4p[:, :, :])
            nc.gpsimd.collective_compute(
                "AllReduce", mybir.AluOpType.add, replica_groups=groups_all,
                ins=[z4_part[:, :, :]], outs=[z4_sum[:, :, :]])
            with tc.tile_pool(name="phFo", bufs=1) as fopool, \
                 tc.tile_pool(name="psFo", bufs=1, space="PSUM") as psFo:
                fc2_b = fopool.tile([128, 2], f32)
                nc.sync.dma_start(fc2_b[:], fc2_b_in[:])
                z4s = fopool.tile([128, 2, B], f32)
                nc.sync.dma_start(z4s[:, :, :], z4_sum[:, :, :])
                z4T = fopool.tile([128, 2, B], bf16)
                for mt in range(2):
                    nc.scalar.activation(z4T[:, mt, :], z4s[:, mt, :], Act.Relu,
                                         bias=fc2_b[:, mt:mt + 1])
                out_W = fopool.tile([128, 3, 1], bf16)
                nc.sync.dma_start(out_W[:, :, :], out_W_in[:, :, :])
                out_b = fopool.tile([1, 1], f32)
                nc.sync.dma_start(out_b[:], out_b_in[:])
                pso = psFo.tile([1, B], f32, tag="po")
                rhs_o = [z4T[:, 0, :], z4T[:, 1, :], xtcT]
                for kt in range(3):
                    kf = 10 if kt == 2 else 128
                    nc.tensor.matmul(pso[:], out_W[0:kf, kt, :], rhs_o[kt],
                                     start=(kt == 0), stop=(kt == 2))
                fin = fopool.tile([1, B], f32)
                nc.vector.scalar_tensor_tensor(fin[:], pso[:], 1.0, out_b[:]
                                               .broadcast_to((1, B)),
                                               AluOp.mult, AluOp.add)
                nc.vector.tensor_scalar(fin[:], fin[:], 100.0, -100.0,
                                        AluOp.min, AluOp.max)
                nc.sync.dma_start(out_t[:, :].rearrange("b o -> o b"), fin[:])

    nc.compile()
    return nc


def kernel(**inputs):
    in_maps, struct = _build_host_inputs(inputs)
    nc = _build_program(struct)
    import os
    trace = bool(int(os.environ.get("GNN_TRACE", "0")))
    res = run_bass_kernel_spmd(nc, in_maps, core_ids=list(range(NCORES)), trace=trace)
    kernel.last_result = res
    return np.asarray(res.results[0]["out"]).reshape(B, 1).astype(np.float32)


# revision 23
# speedup vs baseline: 1.2533x; 1.2533x over previous
"""Bass/Trainium2 kernel for nn_GATGCNNet (GAT conv -> GCN conv -> pool -> MLPs, two drug
branches + cell-line MLP fusion), distributed over 8 NeuronCores.

Sharding: cores 0-3 = drug branch 1, cores 4-7 = drug branch 2; within a branch each core
owns a contiguous 5120-node / 128-graph shard (dst-sharded edges).

Key structure (v2):
- GAT is factorized through its linearity: out_h[dst] = (sum_src alpha_h * x[src]) @ W_h,
  so the per-edge gather reads raw features x (77) + a_s (10) packed in 256B bf16 rows of a
  host-precomputed table that is replicated per core -> the first AllGather is eliminated.
- The GCN-phase node table (x1 rows) is bf16 and its AllGather is split into 4 row-quarters
  issued as phase B produces them, overlapping the collective with compute.
- All matmuls run in bf16 (f32 PSUM accumulation); biases are folded into the contraction
  as an extra all-ones row.
"""

import numpy as np
from ml_dtypes import bfloat16

import concourse.bass as bass
import concourse.bacc as bacc
import concourse.mybir as mybir
import concourse.tile as tile
from concourse import library_config
from concourse.bass_utils import run_bass_kernel_spmd

# problem constants (hardcoded per the task contract)
H, F, HF = 10, 77, 770
B, N, E = 512, 20480, 81920
NCORES = 8
R = 4                 # cores per branch
NSH = N // R          # nodes per core (5120)
NGRP = NSH // 128     # 128-node groups per core (40)
NPG = N // B          # nodes per graph (40)
GPC = B // R          # graphs per core-shard (128)
NQ = 8                # row-slices for the pipelined x1-table allgather
QGRP = NGRP // NQ     # groups per slice (5)
QROWS = 128 * QGRP    # rows per slice (640)
T1W = 128             # bf16 T1 row: [x 0:77 | a_s 77:87 | pad]  (256B)
T2W = 896             # bf16 T2 row: [x1 0:770 | pad]            (1792B)

f32 = mybir.dt.float32
bf16 = mybir.dt.bfloat16
i16 = mybir.dt.int16


def _wrap_idx(a):
    """[n] int array (n % 16 == 0) -> [128, n/16] int16 wrapped layout, replicated x8."""
    w = a.astype(np.int16).reshape(-1, 16).T
    return np.tile(w, (8, 1))


def _pack_k(w, nkt, m, dtype=bfloat16):
    """Pad [K, M] -> [128*nkt, M] with zeros, return [128, nkt, M] (row 128*kt+p at [p, kt])."""
    w = np.asarray(w, np.float32)
    kp = np.zeros((128 * nkt, m), np.float32)
    kp[: w.shape[0]] = w
    return np.ascontiguousarray(kp.reshape(nkt, 128, m).transpose(1, 0, 2)).astype(dtype)


def _bias_cols(b, nm):
    """bias [M] -> [128, nm] f32 with tile m's bias in column m (padded)."""
    bp = np.zeros(128 * nm, np.float32)
    bp[: len(b)] = b
    return np.ascontiguousarray(bp.reshape(nm, 128).T)


def _fcg1_pack(w):
    """fcg1_W [1540, 1500] -> [128, 14, 1536] bf16: kt0-6 = mx rows, kt7-13 = mean rows."""
    out = np.zeros((14, 128, 1536), np.float32)
    for j, base in ((0, 0), (7, HF)):
        for t in range(7):
            lo = base + 128 * t
            hi = min(base + HF, lo + 128)
            if hi > lo:
                out[j + t, : hi - lo, :1500] = w[lo:hi]
    return np.ascontiguousarray(out.transpose(1, 0, 2)).astype(bfloat16)


def _t2_row_of(n):
    """Global node id -> row in the quarter-blocked T2_full layout."""
    c, l = n // NSH, n % NSH
    q = l // QROWS
    return R * QROWS * q + QROWS * c + (l % QROWS)


def _edge_structure(edge_lists):
    """Max per-group chunk count across all (branch, core) entries (shared => SPMD)."""
    cnts = np.zeros((len(edge_lists), NGRP), np.int64)
    for i, (srcs, dstl) in enumerate(edge_lists):
        cnts[i] = np.bincount(dstl // 128, minlength=NGRP)
    C = np.maximum(1, np.ceil(cnts.max(axis=0) / 128).astype(np.int64))
    return C


def _build_host_inputs(inputs):
    per_branch = {}
    edge_lists = []
    for d in ("1", "2"):
        ei = np.asarray(inputs["edge_index" + d])
        src = np.concatenate([ei[0], np.arange(N, dtype=ei.dtype)]).astype(np.int64)
        dst = np.concatenate([ei[1], np.arange(N, dtype=ei.dtype)]).astype(np.int64)
        order = np.argsort(dst, kind="stable")
        src, dst = src[order], dst[order]
        deg = np.bincount(dst, minlength=N).astype(np.float32)
        dinv = 1.0 / np.sqrt(np.maximum(deg, 1.0))
        batch = np.asarray(inputs["batch_d" + d])
        assert np.array_equal(batch, np.repeat(np.arange(B, dtype=batch.dtype), NPG)), \
            "kernel assumes uniform contiguous 40-node graphs"

        # host precompute: attention projections a_s/a_d for all nodes (O(N*20))
        x = np.asarray(inputs["xd" + d], np.float32)
        gat_W = np.asarray(inputs["gat_W" + d], np.float32)            # [77, 770]
        att_s = np.asarray(inputs["gat_as" + d], np.float32)            # [10, 77]
        att_d = np.asarray(inputs["gat_ad" + d], np.float32)
        WA = np.zeros((77, 2 * H), np.float32)
        for h in range(H):
            Wh = gat_W[:, h * F:(h + 1) * F]
            WA[:, h] = Wh @ att_s[h]
            WA[:, H + h] = Wh @ att_d[h]
        a_sd = x @ WA                                                   # [N, 20]

        # T1 table: [x | 1 | pad] bf16, 256B rows, replicated per branch core
        T1 = np.zeros((N, T1W), np.float32)
        T1[:, 0:77] = x
        T1[:, 77] = 1.0
        per_branch[d] = dict(src=src, dst=dst, dinv=dinv,
                             T1=np.ascontiguousarray(T1).astype(bfloat16),
                             a_s=a_sd[:, 0:H], a_d=a_sd[:, H:2 * H])
        for r in range(R):
            lo, hi = NSH * r, NSH * (r + 1)
            m = (dst >= lo) & (dst < hi)
            edge_lists.append((src[m], dst[m] - lo))

    C = _edge_structure(edge_lists)
    NCHUNK = int(C.sum())
    TOTCOL = 8 * NCHUNK
    group_cols = np.zeros(NGRP, np.int64)
    acc = 0
    for g in range(NGRP):
        group_cols[g] = acc
        acc += 8 * int(C[g])
    amax = max(float(np.max(per_branch[d]["a_s"]) + np.max(per_branch[d]["a_d"]))
               for d in ("1", "2"))
    struct = dict(C=[int(c) for c in C], group_cols=[int(c) for c in group_cols],
                  NCHUNK=NCHUNK, TOTCOL=TOTCOL, need_clamp=bool(amax >= 80.0))

    t2row = _t2_row_of(np.arange(N))

    core_edge = {}
    for ci in range(NCORES):
        d = "1" if ci < R else "2"
        r = ci % R
        srcs, dstl = edge_lists[(0 if ci < R else R) + r]
        gids = dstl // 128
        idx_b = np.zeros((128, TOTCOL), np.int16)
        idx_c = np.zeros((128, TOTCOL), np.int16)
        ld_col = np.full((128, NCHUNK), 255.0, np.float32)
        adc = np.zeros((128, NCHUNK, H), np.float32)   # a_s[src_e]+a_d[dst_e] per edge
        a_s = per_branch[d]["a_s"]
        a_d = per_branch[d]["a_d"]
        kbase = 0
        for g in range(NGRP):
            m = gids == g
            gs = srcs[m]
            gd = dstl[m] + NSH * r                      # global dst id
            gl = dstl[m] - 128 * g
            cap = 128 * int(C[g])
            padn = cap - len(gs)
            gs_p = np.concatenate([gs, np.zeros(padn, np.int64)])
            gd_p = np.concatenate([gd, np.zeros(padn, np.int64)])
            gl_p = np.concatenate([gl, np.full(padn, 255, np.int64)])
            cc = int(group_cols[g])
            idx_b[:, cc:cc + 8 * int(C[g])] = _wrap_idx(gs_p)
            idx_c[:, cc:cc + 8 * int(C[g])] = _wrap_idx(t2row[gs_p])
            for k in range(int(C[g])):
                ld_col[:, kbase + k] = gl_p[k * 128:(k + 1) * 128]
                adc[:, kbase + k, :] = (a_s[gs_p[k * 128:(k + 1) * 128]]
                                        + a_d[gd_p[k * 128:(k + 1) * 128]])
            kbase += int(C[g])
        pb = per_branch[d]
        dinv_sh = pb["dinv"][NSH * r:NSH * (r + 1)]
        core_edge[ci] = dict(
            idx_b=idx_b, idx_c=idx_c, ld=ld_col,
            dinv=np.ascontiguousarray(dinv_sh.reshape(NGRP, 128).T),
            adc=adc.astype(bfloat16),
        )

    wmaps = {}
    for d in ("1", "2"):
        gat_W = np.asarray(inputs["gat_W" + d], np.float32)
        Wp77 = np.zeros((128, H, F), np.float32)                       # per-head blocks
        for h in range(H):
            Wp77[0:77, h, :] = gat_W[:, h * F:(h + 1) * F]
        wmaps[d] = dict(
            Wp77=Wp77.astype(bfloat16),                                # [128,10,77] bf16
            gcn_W_p=_pack_k(np.asarray(inputs["gcn_W" + d], np.float32), 7, HF),
            gat_brow=np.asarray(inputs["gat_b" + d], np.float32)
                .reshape(1, HF).astype(bfloat16),
            gcn_brow=np.asarray(inputs["gcn_b" + d], np.float32)
                .reshape(1, HF).astype(bfloat16),
            fcg1_W_p=_fcg1_pack(np.asarray(inputs["fcg1_W" + d], np.float32)),
            fcg1_b=_bias_cols(np.asarray(inputs["fcg1_b" + d], np.float32), 12),
        )

    fcg2_W_p = _pack_k(np.asarray(inputs["fcg2_W"], np.float32), 12, 128)
    fcg2_b = _bias_cols(np.asarray(inputs["fcg2_b"], np.float32), 1)
    xcT = np.concatenate([inputs["xc1"], inputs["xc2"], inputs["xc3"], inputs["xtc"]],
                         axis=1).astype(np.float32).T                   # [1546, 512]
    xcT_p = _pack_k(xcT, 13, B)
    cl_W1 = np.asarray(inputs["cl_W1"], np.float32)
    cl_b1 = np.asarray(inputs["cl_b1"], np.float32)
    cl_W2 = np.asarray(inputs["cl_W2"], np.float32)
    cl_b2 = _bias_cols(np.asarray(inputs["cl_b2"], np.float32), 2)
    fc1_W = np.asarray(inputs["fc1_W"], np.float32)
    fc1_b = np.asarray(inputs["fc1_b"], np.float32)
    fc2_W = np.asarray(inputs["fc2_W"], np.float32)
    fc2_b = _bias_cols(np.asarray(inputs["fc2_b"], np.float32), 2)
    out_W_p = _pack_k(np.asarray(inputs["out_W"], np.float32), 3, 1)
    out_b = np.asarray(inputs["out_b"], np.float32).reshape(1, 1)

    iota_row = np.tile(np.arange(128, dtype=np.float32)[None, :], (128, 1)).astype(bfloat16)
    ident = np.eye(128, dtype=np.float32).astype(bfloat16)

    in_maps = []
    for ci in range(NCORES):
        d = "1" if ci < R else "2"
        w = wmaps[d]
        ce = core_edge[ci]
        in_maps.append(dict(
            T1=per_branch[d]["T1"],
            Wp77=w["Wp77"], gcn_W_p=w["gcn_W_p"],
            gat_brow=w["gat_brow"], gcn_brow=w["gcn_brow"],
            fcg1_W_p=w["fcg1_W_p"], fcg1_b=w["fcg1_b"],
            fcg2_W_p=fcg2_W_p, fcg2_b=fcg2_b,
            xcT_p=xcT_p, cl_b2=cl_b2, fc2_b=fc2_b,
            cl_W1s=_pack_k(cl_W1[:, 128 * ci:128 * (ci + 1)], 13, 128),
            cl_b1s=_bias_cols(cl_b1[128 * ci:128 * (ci + 1)], 1),
            cl_W2s=cl_W2[128 * ci:128 * (ci + 1), :].astype(bfloat16),
            fc1_Ws=_pack_k(fc1_W[:, 256 * ci:256 * (ci + 1)], 5, 256),
            fc1_bs=_bias_cols(fc1_b[256 * ci:256 * (ci + 1)], 2),
            fc2_Ws=_pack_k(fc2_W[256 * ci:256 * (ci + 1), :], 2, 256),
            out_W_p=out_W_p, out_b=out_b,
            idx_b=ce["idx_b"], idx_c=ce["idx_c"], ld=ce["ld"], dinv=ce["dinv"],
            adc=ce["adc"], iota=iota_row, ident=ident,
        ))
    return in_maps, struct


def _build_program(struct):
    C = struct["C"]
    group_cols = struct["group_cols"]
    NCHUNK = struct["NCHUNK"]
    TOTCOL = struct["TOTCOL"]
    need_clamp = struct.get("need_clamp", True)
    AluOp = mybir.AluOpType
    Act = mybir.ActivationFunctionType

    nc = bacc.Bacc("TRN2", target_bir_lowering=False, debug=False, num_devices=NCORES)

    # --- inputs ---
    T1_in = nc.dram_tensor("T1", [N, T1W], bf16, kind="ExternalInput")
    Wp77_in = nc.dram_tensor("Wp77", [128, H, F], bf16, kind="ExternalInput")
    gcn_W_in = nc.dram_tensor("gcn_W_p", [128, 7, HF], bf16, kind="ExternalInput")
    gat_brow_in = nc.dram_tensor("gat_brow", [1, HF], bf16, kind="ExternalInput")
    gcn_brow_in = nc.dram_tensor("gcn_brow", [1, HF], bf16, kind="ExternalInput")
    fcg1_W_in = nc.dram_tensor("fcg1_W_p", [128, 14, 1536], bf16, kind="ExternalInput")
    fcg1_b_in = nc.dram_tensor("fcg1_b", [128, 12], f32, kind="ExternalInput")
    fcg2_W_in = nc.dram_tensor("fcg2_W_p", [128, 12, 128], bf16, kind="ExternalInput")
    fcg2_b_in = nc.dram_tensor("fcg2_b", [128, 1], f32, kind="ExternalInput")
    xcT_in = nc.dram_tensor("xcT_p", [128, 13, B], bf16, kind="ExternalInput")
    cl_W1s_in = nc.dram_tensor("cl_W1s", [128, 13, 128], bf16, kind="ExternalInput")
    cl_b1s_in = nc.dram_tensor("cl_b1s", [128, 1], f32, kind="ExternalInput")
    cl_W2s_in = nc.dram_tensor("cl_W2s", [128, 256], bf16, kind="ExternalInput")
    cl_b2_in = nc.dram_tensor("cl_b2", [128, 2], f32, kind="ExternalInput")
    fc1_Ws_in = nc.dram_tensor("fc1_Ws", [128, 5, 256], bf16, kind="ExternalInput")
    fc1_bs_in = nc.dram_tensor("fc1_bs", [128, 2], f32, kind="ExternalInput")
    fc2_Ws_in = nc.dram_tensor("fc2_Ws", [128, 2, 256], bf16, kind="ExternalInput")
    fc2_b_in = nc.dram_tensor("fc2_b", [128, 2], f32, kind="ExternalInput")
    out_W_in = nc.dram_tensor("out_W_p", [128, 3, 1], bf16, kind="ExternalInput")
    out_b_in = nc.dram_tensor("out_b", [1, 1], f32, kind="ExternalInput")
    idx_b_in = nc.dram_tensor("idx_b", [128, TOTCOL], i16, kind="ExternalInput")
    idx_c_in = nc.dram_tensor("idx_c", [128, TOTCOL], i16, kind="ExternalInput")
    ld_in = nc.dram_tensor("ld", [128, NCHUNK], f32, kind="ExternalInput")
    dinv_in = nc.dram_tensor("dinv", [128, NGRP], f32, kind="ExternalInput")
    adc_in = nc.dram_tensor("adc", [128, NCHUNK, H], bf16, kind="ExternalInput")
    iota_in = nc.dram_tensor("iota", [128, 128], bf16, kind="ExternalInput")
    ident_in = nc.dram_tensor("ident", [128, 128], bf16, kind="ExternalInput")

    out_t = nc.dram_tensor("out", [B, 1], f32, kind="ExternalOutput")

    # --- internal DRAM ---
    T2_loc_q = [nc.dram_tensor(f"T2_loc_q{q}", [QROWS, T2W], bf16) for q in range(NQ)]
    T2_full = nc.dram_tensor("T2_full", [N, T2W], bf16)
    x2T_dram = nc.dram_tensor("x2T_dram", [7 * 128, NSH], bf16)
    Hg_loc = nc.dram_tensor("Hg_loc", [128, GPC], bf16)
    H_full = nc.dram_tensor("H_full", [128 * NCORES, GPC], bf16, addr_space="Shared")
    xcl_part = nc.dram_tensor("xcl_part", [128, 2, B], f32)
    xcl_sum = nc.dram_tensor("xcl_sum", [128, 2, B], f32)
    z4_part = nc.dram_tensor("z4_part", [128, 2, B], f32)
    z4_sum = nc.dram_tensor("z4_sum", [128, 2, B], f32)

    groups_b1 = [[0, 1, 2, 3], [4, 5, 6, 7]]
    groups_all = [list(range(NCORES))]

    with tile.TileContext(nc) as tc:
        nc.gpsimd.load_library(library_config.mlp)
        with tc.tile_pool(name="const", bufs=1) as cpool:
            iota = cpool.tile([128, 128], bf16)
            nc.sync.dma_start(iota[:], iota_in[:])
            ident = cpool.tile([128, 128], bf16)
            nc.sync.dma_start(ident[:], ident_in[:])
            idx_b = cpool.tile([128, TOTCOL], i16)
            nc.sync.dma_start(idx_b[:], idx_b_in[:])
            idx_c = cpool.tile([128, TOTCOL], i16)
            nc.sync.dma_start(idx_c[:], idx_c_in[:])
            ldc = cpool.tile([128, NCHUNK], f32)
            nc.sync.dma_start(ldc[:], ld_in[:])
            dinv = cpool.tile([128, NGRP], f32)
            nc.sync.dma_start(dinv[:], dinv_in[:])
            adc = cpool.tile([128, NCHUNK, H], bf16)
            nc.sync.dma_start(adc[:, :, :], adc_in[:, :, :])
            Wp77 = cpool.tile([128, H, F], bf16)
            nc.sync.dma_start(Wp77[:, :, :], Wp77_in[:, :, :])
            gat_brow = cpool.tile([1, HF], bf16)
            nc.sync.dma_start(gat_brow[:], gat_brow_in[:])
            gcn_brow = cpool.tile([1, HF], bf16)
            nc.sync.dma_start(gcn_brow[:], gcn_brow_in[:])
            ones1 = cpool.tile([1, 128], bf16)
            nc.vector.memset(ones1[:], 1.0)
            xcT = cpool.tile([128, 13, B], bf16)
            nc.sync.dma_start(xcT[:, :, :], xcT_in[:, :, :])

            # -------- cell-line MLP, feature-sharded; partial sum AllReduced --------
            # (runs early on the tensor engine; the AllReduce overlaps phase B)
            with tc.tile_pool(name="phCL", bufs=1) as clp, \
                 tc.tile_pool(name="psCL", bufs=1, space="PSUM") as psCL:
                cl_W1s = clp.tile([128, 13, 128], bf16)
                nc.sync.dma_start(cl_W1s[:, :, :], cl_W1s_in[:, :, :])
                cl_b1s = clp.tile([128, 1], f32)
                nc.sync.dma_start(cl_b1s[:], cl_b1s_in[:])
                cl_W2s = clp.tile([128, 256], bf16)
                nc.sync.dma_start(cl_W2s[:], cl_W2s_in[:])
                psz = psCL.tile([128, B], f32, tag="z")
                for kt in range(13):
                    nc.tensor.matmul(psz[:], cl_W1s[:, kt, :], xcT[:, kt, :],
                                     start=(kt == 0), stop=(kt == 12))
                zc1 = clp.tile([128, B], bf16)
                nc.scalar.activation(zc1[:], psz[:], Act.Relu, bias=cl_b1s[:, 0:1])
                xclp = clp.tile([128, 2, B], f32)
                for mt in range(2):
                    psz2 = psCL.tile([128, B], f32, tag="z2")
                    nc.tensor.matmul(psz2[:], cl_W2s[:, 128 * mt:128 * (mt + 1)],
                                     zc1[:], start=True, stop=True)
                    nc.scalar.activation(xclp[:, mt, :], psz2[:], Act.Copy)
                nc.sync.dma_start(xcl_part[:, :, :], xclp[:, :, :])
            nc.gpsimd.collective_compute(
                "AllReduce", mybir.AluOpType.add, replica_groups=groups_all,
                ins=[xcl_part[:, :, :]], outs=[xcl_sum[:, :, :]])

            # ---------------- Phase B: GAT conv (factorized) ----------------
            with tc.tile_pool(name="phB", bufs=3) as bpool, \
                 tc.tile_pool(name="phBsk", bufs=3) as skpool, \
                 tc.tile_pool(name="phBs", bufs=3) as spool, \
                 tc.tile_pool(name="phBt", bufs=2) as tpool, \
                 tc.tile_pool(name="psB", bufs=2, space="PSUM") as psB, \
                 tc.tile_pool(name="psBt", bufs=1, space="PSUM") as psT, \
                 tc.tile_pool(name="psBx", bufs=1, space="PSUM") as psX:
                kbase = 0
                for g in range(NGRP):
                    Cg = C[g]
                    cc = group_cols[g]
                    G = bpool.tile([128, Cg, T1W], bf16, tag="G")
                    nc.gpsimd.dma_gather(
                        G[:, :, :], T1_in[:, :], idx_b[:, cc:cc + 8 * Cg],
                        128 * Cg, 128 * Cg, T1W)
                    # alpha -> e for all chunks of the group at once
                    al = spool.tile([128, Cg, H], f32, tag="al")
                    asl = adc[:, kbase:kbase + Cg, :]
                    nc.vector.scalar_tensor_tensor(al[:, :, :], asl, 0.2,
                                                   asl, AluOp.mult, AluOp.max)
                    if need_clamp:
                        nc.vector.tensor_scalar(al[:, :, :], al[:, :, :], 60.0, None,
                                                AluOp.min)
                    ee = spool.tile([128, Cg, H], bf16, tag="ee")
                    nc.scalar.activation(ee[:, :, :], al[:, :, :], Act.Exp)
                    # select matrices for all chunks in one op
                    Sk_all = skpool.tile([128, Cg, 128], bf16, tag="Sk")
                    nc.vector.tensor_tensor(
                        Sk_all[:, :, :],
                        ldc[:, kbase:kbase + Cg].rearrange("p (c o) -> p c o", o=1)
                            .broadcast_to((128, Cg, 128)),
                        iota[:, :].rearrange("p (o j) -> p o j", o=1)
                            .broadcast_to((128, Cg, 128)),
                        AluOp.is_equal)
                    # weighted messages [e*x | e] per head (78 cols each), one 4D op
                    Gw = spool.tile([128, Cg, HF + H], bf16, tag="Gw")
                    e4 = ee[:, :, :].rearrange("p c (h o) -> p c h o", o=1) \
                        .broadcast_to((128, Cg, H, F + 1))
                    x4 = G[:, :, 0:78].rearrange("p c (o f) -> p c o f", o=1) \
                        .broadcast_to((128, Cg, H, F + 1))
                    nc.vector.tensor_tensor(
                        Gw[:, :, :].rearrange("p c (h f) -> p c h f", f=F + 1),
                        e4, x4, AluOp.mult)
                    pg = psB.tile([128, HF + H], f32, tag="agg")
                    for k in range(Cg):
                        nc.tensor.matmul(pg[:, 0:512], Sk_all[:, k, :], Gw[:, k, 0:512],
                                         start=(k == 0), stop=(k == Cg - 1))
                        nc.tensor.matmul(pg[:, 512:HF + H], Sk_all[:, k, :],
                                         Gw[:, k, 512:HF + H],
                                         start=(k == 0), stop=(k == Cg - 1))
                    kbase += Cg
                    # normalize, transpose, apply block-diag W (+bias row), ELU, *dinv
                    pgs = spool.tile([128, HF + H], f32, tag="pgs")
                    nc.scalar.activation(pgs[:], pg[:], Act.Copy)
                    pg4 = pgs[:].rearrange("p (h f) -> p h f", f=F + 1)
                    rden = spool.tile([128, H], f32, tag="rden")
                    nc.vector.reciprocal(rden[:], pg4[:, :, F:F + 1]
                                         .rearrange("p h o -> p (h o)"))
                    aggn = spool.tile([128, HF], bf16, tag="aggn")
                    r3 = rden[:].rearrange("p (h o) -> p h o", o=1) \
                        .broadcast_to((128, H, F))
                    nc.vector.tensor_tensor(
                        aggn[:].rearrange("p (h f) -> p h f", f=F),
                        pg4[:, :, 0:F], r3, AluOp.mult)
                    aggnT = tpool.tile([128, H, 128], bf16, tag="aT")
                    ptb = psT.tile([128, H, 128], bf16, tag="tr")
                    for h in range(H):
                        nc.tensor.transpose(ptb[0:77, h, :],
                                            aggn[:, F * h:F * (h + 1)], ident[:])
                    nc.scalar.activation(aggnT[:, :, :], ptb[:, :, :], Act.Copy)
                    # W output: heads 0-5 at cols 77h (bank 0), heads 6-9 at
                    # 512+77(h-6) (bank 1) -- no matmul crosses a PSUM bank.
                    # Bias is accumulated via a K=1 all-ones matmul per head.
                    ps_x1 = psX.tile([128, 1024], f32, tag="x1")
                    for h in range(H):
                        off = 77 * h if h < 6 else 512 + 77 * (h - 6)
                        nc.tensor.matmul(ps_x1[:, off:off + F], aggnT[0:77, h, :],
                                         Wp77[0:77, h, :], start=True, stop=False)
                        nc.tensor.matmul(ps_x1[:, off:off + F], ones1[0:1, :],
                                         gat_brow[0:1, F * h:F * (h + 1)],
                                         start=False, stop=True)
                    m = spool.tile([128, 1024], bf16, tag="m")
                    nc.vector.tensor_scalar(m[:], ps_x1[:], 0.0, None, AluOp.min)
                    e2 = spool.tile([128, 1024], bf16, tag="e2")
                    nc.scalar.activation(e2[:], m[:], Act.Exp)
                    x1f = spool.tile([128, 1024], f32, tag="x1f")
                    nc.vector.scalar_tensor_tensor(x1f[:], e2[:], -1.0, ps_x1[:],
                                                   AluOp.add, AluOp.max)
                    T2row = bpool.tile([128, T2W], bf16, tag="T2row")
                    nc.scalar.activation(T2row[:, 0:462], x1f[:, 0:462], Act.Copy,
                                         scale=dinv[:, g:g + 1])
                    nc.scalar.activation(T2row[:, 462:HF], x1f[:, 512:820], Act.Copy,
                                         scale=dinv[:, g:g + 1])
                    q, j = g // QGRP, g % QGRP
                    nc.sync.dma_start(T2_loc_q[q][128 * j:128 * (j + 1), 0:HF], T2row[:, 0:HF])
                    if j == QGRP - 1:
                        nc.gpsimd.collective_compute(
                            "AllGather", mybir.AluOpType.bypass,
                            replica_groups=groups_b1,
                            ins=[T2_loc_q[q][:, :]],
                            outs=[T2_full[R * QROWS * q:R * QROWS * (q + 1), :]])

            gcn_W = cpool.tile([128, 7, HF], bf16)
            nc.sync.dma_start(gcn_W[:, :, :], gcn_W_in[:, :, :])

            # ---------------- Phase C: GCN conv ----------------
            # Aggregate per 128-dst group; batch 4 groups into a 512-node super-group and
            # produce x2T directly: x2T[fo, n] = relu(sum_fi W[fi, fo] * aggT[fi, n] + b).
            with tc.tile_pool(name="phC", bufs=2) as cpool2, \
                 tc.tile_pool(name="phCsk", bufs=2) as skpool2, \
                 tc.tile_pool(name="phCs", bufs=2) as cspool, \
                 tc.tile_pool(name="phCt", bufs=2) as tpool2, \
                 tc.tile_pool(name="psC", bufs=2, space="PSUM") as psC, \
                 tc.tile_pool(name="psCt", bufs=2, space="PSUM") as psCt, \
                 tc.tile_pool(name="psCx", bufs=2, space="PSUM") as psCx:
                onesd = cpool.tile([1, 512], bf16)
                nc.vector.memset(onesd[:], 1.0)
                kbase = 0
                for sg in range(NGRP // 4):
                    aggT_sg = tpool2.tile([128, 7, 512], bf16, tag="aggT")
                    for gi in range(4):
                        g = 4 * sg + gi
                        Cg = C[g]
                        cc = group_cols[g]
                        G2 = cpool2.tile([128, Cg, T2W], bf16, tag="G2")
                        nc.gpsimd.dma_gather(
                            G2[:, :, :], T2_full[:, :], idx_c[:, cc:cc + 8 * Cg],
                            128 * Cg, 128 * Cg, T2W)
                        Sk_all = skpool2.tile([128, Cg, 128], bf16, tag="S2")
                        nc.vector.tensor_tensor(
                            Sk_all[:, :, :],
                            ldc[:, kbase:kbase + Cg].rearrange("p (c o) -> p c o", o=1)
                                .broadcast_to((128, Cg, 128)),
                            iota[:, :].rearrange("p (o j) -> p o j", o=1)
                                .broadcast_to((128, Cg, 128)),
                            AluOp.is_equal)
                        pg = psC.tile([128, HF], f32, tag="acc2")
                        for k in range(Cg):
                            nc.tensor.matmul(pg[:, 0:512], Sk_all[:, k, :],
                                             G2[:, k, 0:512],
                                             start=(k == 0), stop=(k == Cg - 1))
                            nc.tensor.matmul(pg[:, 512:HF], Sk_all[:, k, :],
                                             G2[:, k, 512:HF],
                                             start=(k == 0), stop=(k == Cg - 1))
                            kbase += 1
                        agg = cspool.tile([128, HF], bf16, tag="agg")
                        nc.scalar.activation(agg[:], pg[:], Act.Copy,
                                             scale=dinv[:, g:g + 1])
                        ptb = psCt.tile([128, 7, 128], bf16, tag="tr")
                        for ft in range(7):
                            kf = min(128, HF - 128 * ft)
                            nc.tensor.transpose(ptb[0:kf, ft, :],
                                                agg[:, 128 * ft:128 * ft + kf], ident[:])
                        nc.scalar.activation(aggT_sg[:, :, 128 * gi:128 * (gi + 1)],
                                             ptb[:, :, :], Act.Copy)
                    x2T_sb = cspool.tile([128, 7, 512], bf16, tag="x2T")
                    for mt in range(7):
                        mm = min(128, HF - 128 * mt)
                        psx = psCx.tile([128, 512], f32, tag="x2ps")
                        for kt in range(7):
                            kf = min(128, HF - 128 * kt)
                            nc.tensor.matmul(psx[0:mm, :],
                                             gcn_W[0:kf, kt, 128 * mt:128 * mt + mm],
                                             aggT_sg[0:kf, kt, :],
                                             start=(kt == 0), stop=False)
                        nc.tensor.matmul(psx[0:mm, :],
                                         gcn_brow[0:1, 128 * mt:128 * mt + mm],
                                         onesd[0:1, :], start=False, stop=True)
                        nc.scalar.activation(x2T_sb[0:mm, mt, :], psx[0:mm, :], Act.Relu)
                        nc.sync.dma_start(
                            x2T_dram[128 * mt:128 * mt + mm, 512 * sg:512 * (sg + 1)],
                            x2T_sb[0:mm, mt, :])

            # ---------------- Phase D: pooling ----------------
            with tc.tile_pool(name="pooled", bufs=1) as plpool:
                pooled = plpool.tile([128, 14, 128], bf16)
                nc.vector.memset(pooled[:, 6, :], 0.0)
                nc.vector.memset(pooled[:, 13, :], 0.0)
                with tc.tile_pool(name="phD", bufs=2) as dpool:
                    for ft in range(7):
                        kf = min(128, HF - 128 * ft)
                        strip = dpool.tile([128, NSH], bf16, tag="strip")
                        nc.sync.dma_start(strip[0:kf, :],
                                          x2T_dram[128 * ft:128 * ft + kf, :])
                        seg = strip[0:kf, :].rearrange("p (gr n) -> p gr n", n=NPG)
                        nc.vector.tensor_reduce(pooled[0:kf, ft, :], seg,
                                                mybir.AxisListType.X, AluOp.max)
                        sm = dpool.tile([128, 128], f32, tag="sm")
                        nc.vector.tensor_reduce(sm[0:kf, :], seg,
                                                mybir.AxisListType.X, AluOp.add)
                        nc.vector.tensor_scalar(pooled[0:kf, 7 + ft, :], sm[0:kf, :],
                                                1.0 / NPG, None, AluOp.mult)

                # ---------------- Phase E: fcg1 / fcg2 ----------------
                with tc.tile_pool(name="phE", bufs=1) as epool, \
                     tc.tile_pool(name="psE", bufs=2, space="PSUM") as psE:
                    fcg1_W = epool.tile([128, 14, 1536], bf16)
                    nc.sync.dma_start(fcg1_W[:, :, :], fcg1_W_in[:, :, :])
                    fcg1_b = epool.tile([128, 12], f32)
                    nc.sync.dma_start(fcg1_b[:], fcg1_b_in[:])
                    fcg2_W = epool.tile([128, 12, 128], bf16)
                    nc.sync.dma_start(fcg2_W[:, :, :], fcg2_W_in[:, :, :])
                    fcg2_b = epool.tile([128, 1], f32)
                    nc.sync.dma_start(fcg2_b[:], fcg2_b_in[:])
                    z1T = epool.tile([128, 12, 128], bf16)
                    for mt in range(12):
                        psz = psE.tile([128, 128], f32, tag="ze")
                        for kt in range(14):
                            nc.tensor.matmul(psz[:], fcg1_W[:, kt, 128 * mt:128 * (mt + 1)],
                                             pooled[:, kt, :], start=(kt == 0), stop=(kt == 13))
                        nc.scalar.activation(z1T[:, mt, :], psz[:], Act.Relu,
                                             bias=fcg1_b[:, mt:mt + 1])
                    psh = psE.tile([128, GPC], f32, tag="he")
                    for kt in range(12):
                        nc.tensor.matmul(psh[:], fcg2_W[:, kt, :], z1T[:, kt, :],
                                         start=(kt == 0), stop=(kt == 11))
                    hdT = epool.tile([128, GPC], bf16)
                    nc.scalar.activation(hdT[:], psh[:], Act.Relu, bias=fcg2_b[:, 0:1])
                    nc.sync.dma_start(Hg_loc[:, :], hdT[:])

            nc.gpsimd.collective_compute(
                "AllGather", mybir.AluOpType.bypass, replica_groups=groups_all,
                ins=[Hg_loc[:, :]], outs=[H_full[:, :]])

            # ---------------- Phase F: fusion MLP (feature-sharded) ----------------
            with tc.tile_pool(name="phFa", bufs=1) as fpool, \
                 tc.tile_pool(name="psF", bufs=2, space="PSUM") as psF:
                cl_b2 = fpool.tile([128, 2], f32)
                nc.sync.dma_start(cl_b2[:], cl_b2_in[:])
                xcls = fpool.tile([128, 2, B], f32)
                nc.sync.dma_start(xcls[:, :, :], xcl_sum[:, :, :])
                xclT = fpool.tile([128, 2, B], bf16)
                for mt in range(2):
                    nc.scalar.activation(xclT[:, mt, :], xcls[:, mt, :], Act.Relu,
                                         bias=cl_b2[:, mt:mt + 1])
                h1T = fpool.tile([128, B], bf16)
                h2T = fpool.tile([128, B], bf16)
                for r in range(R):
                    nc.sync.dma_start(h1T[:, GPC * r:GPC * (r + 1)],
                                      H_full[128 * r:128 * (r + 1), :])
                    nc.sync.dma_start(h2T[:, GPC * r:GPC * (r + 1)],
                                      H_full[128 * (R + r):128 * (R + r + 1), :])
                xtcT = xcT[0:10, 12, :]   # xtc rows live at packed rows 1536:1546
                fc1_Ws = fpool.tile([128, 5, 256], bf16)
                nc.sync.dma_start(fc1_Ws[:, :, :], fc1_Ws_in[:, :, :])
                fc1_bs = fpool.tile([128, 2], f32)
                nc.sync.dma_start(fc1_bs[:], fc1_bs_in[:])
                rhs1 = [h1T[:, :], h2T[:, :], xclT[:, 0, :], xclT[:, 1, :], xtcT]
                z3T = fpool.tile([128, 2, B], bf16)
                for mt in range(2):
                    psz = psF.tile([128, B], f32, tag="zf")
                    for kt in range(5):
                        kf = 10 if kt == 4 else 128
                        nc.tensor.matmul(psz[:], fc1_Ws[0:kf, kt, 128 * mt:128 * (mt + 1)],
                                         rhs1[kt], start=(kt == 0), stop=(kt == 4))
                    nc.scalar.activation(z3T[:, mt, :], psz[:], Act.Relu,
                                         bias=fc1_bs[:, mt:mt + 1])
                fc2_Ws = fpool.tile([128, 2, 256], bf16)
                nc.sync.dma_start(fc2_Ws[:, :, :], fc2_Ws_in[:, :, :])
                z4p = fpool.tile([128, 2, B], f32)
                for mt in range(2):
                    psz = psF.tile([128, B], f32, tag="zf")
                    for kt in range(2):
                        nc.tensor.matmul(psz[:], fc2_Ws[:, kt, 128 * mt:128 * (mt + 1)],
                                         z3T[:, kt, :], start=(kt == 0), stop=(kt == 1))
                    nc.scalar.activation(z4p[:, mt, :], psz[:], Act.Copy)
                nc.sync.dma_start(z4_part[:, :, :], z4p[:, :, :])
            nc.gpsimd.collective_compute(
                "AllReduce", mybir.AluOpType.add, replica_groups=groups_all,
                ins=[z4_part[:, :, :]], outs=[z4_sum[:, :, :]])
            with tc.tile_pool(name="phFo", bufs=1) as fopool, \
                 tc.tile_pool(name="psFo", bufs=1, space="PSUM") as psFo:
                fc2_b = fopool.tile([128, 2], f32)
                nc.sync.dma_start(fc2_b[:], fc2_b_in[:])
                z4s = fopool.tile([128, 2, B], f32)
                nc.sync.dma_start(z4s[:, :, :], z4_sum[:, :, :])
                z4T = fopool.tile([128, 2, B], bf16)
                for mt in range(2):
                    nc.scalar.activation(z4T[:, mt, :], z4s[:, mt, :], Act.Relu,
                                         bias=fc2_b[:, mt:mt + 1])
                out_W = fopool.tile([128, 3, 1], bf16)
                nc.sync.dma_start(out_W[:, :, :], out_W_in[:, :, :])
                out_b = fopool.tile([1, 1], f32)
                nc.sync.dma_start(out_b[:], out_b_in[:])
                pso = psFo.tile([1, B], f32, tag="po")
                rhs_o = [z4T[:, 0, :], z4T[:, 1, :], xtcT]
                for kt in range(3):
                    kf = 10 if kt == 2 else 128
                    nc.tensor.matmul(pso[:], out_W[0:kf, kt, :], rhs_o[kt],
                                     start=(kt == 0), stop=(kt == 2))
                fin = fopool.tile([1, B], f32)
                nc.vector.scalar_tensor_tensor(fin[:], pso[:], 1.0, out_b[:]
                                               .broadcast_to((1, B)),
                                               AluOp.mult, AluOp.add)
                nc.vector.tensor_scalar(fin[:], fin[:], 100.0, -100.0,
                                        AluOp.min, AluOp.max)
                nc.sync.dma_start(out_t[:, :].rearrange("b o -> o b"), fin[:])

    nc.compile()
    return nc


def kernel(**inputs):
    in_maps, struct = _build_host_inputs(inputs)
    nc = _build_program(struct)
    import os
    trace = bool(int(os.environ.get("GNN_TRACE", "0")))
    res = run_bass_kernel_spmd(nc, in_maps, core_ids=list(range(NCORES)), trace=trace)
    kernel.last_result = res
    return np.asarray(res.results[0]["out"]).reshape(B, 1).astype(np.float32)


# revision 27
# speedup vs baseline: 1.2549x; 1.0013x over previous
"""Bass/Trainium2 kernel for nn_GATGCNNet (GAT conv -> GCN conv -> pool -> MLPs, two drug
branches + cell-line MLP fusion), distributed over 8 NeuronCores.

Sharding: cores 0-3 = drug branch 1, cores 4-7 = drug branch 2; within a branch each core
owns a contiguous 5120-node / 128-graph shard (dst-sharded edges).

Key structure (v2):
- GAT is factorized through its linearity: out_h[dst] = (sum_src alpha_h * x[src]) @ W_h,
  so the per-edge gather reads raw features x (77) + a_s (10) packed in 256B bf16 rows of a
  host-precomputed table that is replicated per core -> the first AllGather is eliminated.
- The GCN-phase node table (x1 rows) is bf16 and its AllGather is split into 4 row-quarters
  issued as phase B produces them, overlapping the collective with compute.
- All matmuls run in bf16 (f32 PSUM accumulation); biases are folded into the contraction
  as an extra all-ones row.
"""

import numpy as np
from ml_dtypes import bfloat16

import concourse.bass as bass
import concourse.bacc as bacc
import concourse.mybir as mybir
import concourse.tile as tile
from concourse import library_config
from concourse.bass_utils import run_bass_kernel_spmd

# problem constants (hardcoded per the task contract)
H, F, HF = 10, 77, 770
B, N, E = 512, 20480, 81920
NCORES = 8
R = 4                 # cores per branch
NSH = N // R          # nodes per core (5120)
NGRP = NSH // 128     # 128-node groups per core (40)
NPG = N // B          # nodes per graph (40)
GPC = B // R          # graphs per core-shard (128)
NQ = 8                # row-slices for the pipelined x1-table allgather
QGRP = NGRP // NQ     # groups per slice (5)
QROWS = 128 * QGRP    # rows per slice (640)
T1W = 128             # bf16 T1 row: [x 0:77 | a_s 77:87 | pad]  (256B)
T2W = 896             # bf16 T2 row: [x1 0:770 | pad]            (1792B)

f32 = mybir.dt.float32
bf16 = mybir.dt.bfloat16
i16 = mybir.dt.int16


def _wrap_idx(a):
    """[n] int array (n % 16 == 0) -> [128, n/16] int16 wrapped layout, replicated x8."""
    w = a.astype(np.int16).reshape(-1, 16).T
    return np.tile(w, (8, 1))


def _pack_k(w, nkt, m, dtype=bfloat16):
    """Pad [K, M] -> [128*nkt, M] with zeros, return [128, nkt, M] (row 128*kt+p at [p, kt])."""
    w = np.asarray(w, np.float32)
    kp = np.zeros((128 * nkt, m), np.float32)
    kp[: w.shape[0]] = w
    return np.ascontiguousarray(kp.reshape(nkt, 128, m).transpose(1, 0, 2)).astype(dtype)


def _bias_cols(b, nm):
    """bias [M] -> [128, nm] f32 with tile m's bias in column m (padded)."""
    bp = np.zeros(128 * nm, np.float32)
    bp[: len(b)] = b
    return np.ascontiguousarray(bp.reshape(nm, 128).T)


def _fcg1_pack(w):
    """fcg1_W [1540, 1500] -> [128, 14, 1536] bf16: kt0-6 = mx rows, kt7-13 = mean rows."""
    out = np.zeros((14, 128, 1536), np.float32)
    for j, base in ((0, 0), (7, HF)):
        for t in range(7):
            lo = base + 128 * t
            hi = min(base + HF, lo + 128)
            if hi > lo:
                out[j + t, : hi - lo, :1500] = w[lo:hi]
    return np.ascontiguousarray(out.transpose(1, 0, 2)).astype(bfloat16)


def _t2_row_of(n):
    """Global node id -> row in the quarter-blocked T2_full layout."""
    c, l = n // NSH, n % NSH
    q = l // QROWS
    return R * QROWS * q + QROWS * c + (l % QROWS)


def _edge_structure(edge_lists):
    """Max per-group chunk count across all (branch, core) entries (shared => SPMD)."""
    cnts = np.zeros((len(edge_lists), NGRP), np.int64)
    for i, (srcs, dstl) in enumerate(edge_lists):
        cnts[i] = np.bincount(dstl // 128, minlength=NGRP)
    C = np.maximum(1, np.ceil(cnts.max(axis=0) / 128).astype(np.int64))
    return C


def _build_host_inputs(inputs):
    per_branch = {}
    edge_lists = []
    for d in ("1", "2"):
        ei = np.asarray(inputs["edge_index" + d])
        src = np.concatenate([ei[0], np.arange(N, dtype=ei.dtype)]).astype(np.int64)
        dst = np.concatenate([ei[1], np.arange(N, dtype=ei.dtype)]).astype(np.int64)
        order = np.argsort(dst, kind="stable")
        src, dst = src[order], dst[order]
        deg = np.bincount(dst, minlength=N).astype(np.float32)
        dinv = 1.0 / np.sqrt(np.maximum(deg, 1.0))
        batch = np.asarray(inputs["batch_d" + d])
        assert np.array_equal(batch, np.repeat(np.arange(B, dtype=batch.dtype), NPG)), \
            "kernel assumes uniform contiguous 40-node graphs"

        # host precompute: attention projections a_s/a_d for all nodes (O(N*20))
        x = np.asarray(inputs["xd" + d], np.float32)
        gat_W = np.asarray(inputs["gat_W" + d], np.float32)            # [77, 770]
        att_s = np.asarray(inputs["gat_as" + d], np.float32)            # [10, 77]
        att_d = np.asarray(inputs["gat_ad" + d], np.float32)
        WA = np.zeros((77, 2 * H), np.float32)
        for h in range(H):
            Wh = gat_W[:, h * F:(h + 1) * F]
            WA[:, h] = Wh @ att_s[h]
            WA[:, H + h] = Wh @ att_d[h]
        a_sd = x @ WA                                                   # [N, 20]

        # T1 table: [x | 1 | pad] bf16, 256B rows, replicated per branch core
        T1 = np.zeros((N, T1W), np.float32)
        T1[:, 0:77] = x
        T1[:, 77] = 1.0
        per_branch[d] = dict(src=src, dst=dst, dinv=dinv,
                             T1=np.ascontiguousarray(T1).astype(bfloat16),
                             a_s=a_sd[:, 0:H], a_d=a_sd[:, H:2 * H])
        for r in range(R):
            lo, hi = NSH * r, NSH * (r + 1)
            m = (dst >= lo) & (dst < hi)
            edge_lists.append((src[m], dst[m] - lo))

    C = _edge_structure(edge_lists)
    NCHUNK = int(C.sum())
    TOTCOL = 8 * NCHUNK
    group_cols = np.zeros(NGRP, np.int64)
    acc = 0
    for g in range(NGRP):
        group_cols[g] = acc
        acc += 8 * int(C[g])
    amax = max(float(np.max(per_branch[d]["a_s"]) + np.max(per_branch[d]["a_d"]))
               for d in ("1", "2"))
    struct = dict(C=[int(c) for c in C], group_cols=[int(c) for c in group_cols],
                  NCHUNK=NCHUNK, TOTCOL=TOTCOL, need_clamp=bool(amax >= 80.0))

    t2row = _t2_row_of(np.arange(N))

    core_edge = {}
    for ci in range(NCORES):
        d = "1" if ci < R else "2"
        r = ci % R
        srcs, dstl = edge_lists[(0 if ci < R else R) + r]
        gids = dstl // 128
        idx_b = np.zeros((128, TOTCOL), np.int16)
        idx_c = np.zeros((128, TOTCOL), np.int16)
        ld_col = np.full((128, NCHUNK), 255.0, np.float32)
        adc = np.zeros((128, NCHUNK, H), np.float32)   # a_s[src_e]+a_d[dst_e] per edge
        a_s = per_branch[d]["a_s"]
        a_d = per_branch[d]["a_d"]
        kbase = 0
        for g in range(NGRP):
            m = gids == g
            gs = srcs[m]
            gd = dstl[m] + NSH * r                      # global dst id
            gl = dstl[m] - 128 * g
            cap = 128 * int(C[g])
            padn = cap - len(gs)
            gs_p = np.concatenate([gs, np.zeros(padn, np.int64)])
            gd_p = np.concatenate([gd, np.zeros(padn, np.int64)])
            gl_p = np.concatenate([gl, np.full(padn, 255, np.int64)])
            cc = int(group_cols[g])
            idx_b[:, cc:cc + 8 * int(C[g])] = _wrap_idx(gs_p)
            idx_c[:, cc:cc + 8 * int(C[g])] = _wrap_idx(t2row[gs_p])
            for k in range(int(C[g])):
                ld_col[:, kbase + k] = gl_p[k * 128:(k + 1) * 128]
                adc[:, kbase + k, :] = (a_s[gs_p[k * 128:(k + 1) * 128]]
                                        + a_d[gd_p[k * 128:(k + 1) * 128]])
            kbase += int(C[g])
        pb = per_branch[d]
        dinv_sh = pb["dinv"][NSH * r:NSH * (r + 1)]
        core_edge[ci] = dict(
            idx_b=idx_b, idx_c=idx_c, ld=ld_col,
            dinv=np.ascontiguousarray(dinv_sh.reshape(NGRP, 128).T),
            adc=adc.astype(bfloat16),
        )

    wmaps = {}
    for d in ("1", "2"):
        gat_W = np.asarray(inputs["gat_W" + d], np.float32)
        Wp77 = np.zeros((128, H, F), np.float32)                       # per-head blocks
        for h in range(H):
            Wp77[0:77, h, :] = gat_W[:, h * F:(h + 1) * F]
        wmaps[d] = dict(
            Wp77=Wp77.astype(bfloat16),                                # [128,10,77] bf16
            gcn_W_p=_pack_k(np.asarray(inputs["gcn_W" + d], np.float32), 7, HF),
            gat_bh=np.ascontiguousarray(
                np.concatenate([np.asarray(inputs["gat_b" + d], np.float32)
                               .reshape(H, F).T,
                                np.zeros((128 - F, H), np.float32)], axis=0)),
            gcn_brow=np.asarray(inputs["gcn_b" + d], np.float32)
                .reshape(1, HF).astype(bfloat16),
            fcg1_W_p=_fcg1_pack(np.asarray(inputs["fcg1_W" + d], np.float32)),
            fcg1_b=_bias_cols(np.asarray(inputs["fcg1_b" + d], np.float32), 12),
        )

    fcg2_W_p = _pack_k(np.asarray(inputs["fcg2_W"], np.float32), 12, 128)
    fcg2_b = _bias_cols(np.asarray(inputs["fcg2_b"], np.float32), 1)
    xcT = np.concatenate([inputs["xc1"], inputs["xc2"], inputs["xc3"], inputs["xtc"]],
                         axis=1).astype(np.float32).T                   # [1546, 512]
    xcT_p = _pack_k(xcT, 13, B)
    cl_W1 = np.asarray(inputs["cl_W1"], np.float32)
    cl_b1 = np.asarray(inputs["cl_b1"], np.float32)
    cl_W2 = np.asarray(inputs["cl_W2"], np.float32)
    cl_b2 = _bias_cols(np.asarray(inputs["cl_b2"], np.float32), 2)
    fc1_W = np.asarray(inputs["fc1_W"], np.float32)
    fc1_b = np.asarray(inputs["fc1_b"], np.float32)
    fc2_W = np.asarray(inputs["fc2_W"], np.float32)
    fc2_b = _bias_cols(np.asarray(inputs["fc2_b"], np.float32), 2)
    out_W_p = _pack_k(np.asarray(inputs["out_W"], np.float32), 3, 1)
    out_b = np.asarray(inputs["out_b"], np.float32).reshape(1, 1)

    iota_row = np.tile(np.arange(128, dtype=np.float32)[None, :], (128, 1)).astype(bfloat16)
    ident = np.eye(128, dtype=np.float32).astype(bfloat16)

    in_maps = []
    for ci in range(NCORES):
        d = "1" if ci < R else "2"
        w = wmaps[d]
        ce = core_edge[ci]
        in_maps.append(dict(
            T1=per_branch[d]["T1"],
            Wp77=w["Wp77"], gcn_W_p=w["gcn_W_p"],
            gat_bh=w["gat_bh"], gcn_brow=w["gcn_brow"],
            fcg1_W_p=w["fcg1_W_p"], fcg1_b=w["fcg1_b"],
            fcg2_W_p=fcg2_W_p, fcg2_b=fcg2_b,
            xcT_p=xcT_p, cl_b2=cl_b2, fc2_b=fc2_b,
            cl_W1s=_pack_k(cl_W1[:, 128 * ci:128 * (ci + 1)], 13, 128),
            cl_b1s=_bias_cols(cl_b1[128 * ci:128 * (ci + 1)], 1),
            cl_W2s=cl_W2[128 * ci:128 * (ci + 1), :].astype(bfloat16),
            fc1_Ws=_pack_k(fc1_W[:, 256 * ci:256 * (ci + 1)], 5, 256),
            fc1_bs=_bias_cols(fc1_b[256 * ci:256 * (ci + 1)], 2),
            fc2_Ws=_pack_k(fc2_W[256 * ci:256 * (ci + 1), :], 2, 256),
            out_W_p=out_W_p, out_b=out_b,
            idx_b=ce["idx_b"], idx_c=ce["idx_c"], ld=ce["ld"], dinv=ce["dinv"],
            adc=ce["adc"], iota=iota_row, ident=ident,
        ))
    return in_maps, struct


def _build_program(struct):
    C = struct["C"]
    group_cols = struct["group_cols"]
    NCHUNK = struct["NCHUNK"]
    TOTCOL = struct["TOTCOL"]
    need_clamp = struct.get("need_clamp", True)
    AluOp = mybir.AluOpType
    Act = mybir.ActivationFunctionType

    nc = bacc.Bacc("TRN2", target_bir_lowering=False, debug=False, num_devices=NCORES)

    # --- inputs ---
    T1_in = nc.dram_tensor("T1", [N, T1W], bf16, kind="ExternalInput")
    Wp77_in = nc.dram_tensor("Wp77", [128, H, F], bf16, kind="ExternalInput")
    gcn_W_in = nc.dram_tensor("gcn_W_p", [128, 7, HF], bf16, kind="ExternalInput")
    gat_bh_in = nc.dram_tensor("gat_bh", [128, H], f32, kind="ExternalInput")
    gcn_brow_in = nc.dram_tensor("gcn_brow", [1, HF], bf16, kind="ExternalInput")
    fcg1_W_in = nc.dram_tensor("fcg1_W_p", [128, 14, 1536], bf16, kind="ExternalInput")
    fcg1_b_in = nc.dram_tensor("fcg1_b", [128, 12], f32, kind="ExternalInput")
    fcg2_W_in = nc.dram_tensor("fcg2_W_p", [128, 12, 128], bf16, kind="ExternalInput")
    fcg2_b_in = nc.dram_tensor("fcg2_b", [128, 1], f32, kind="ExternalInput")
    xcT_in = nc.dram_tensor("xcT_p", [128, 13, B], bf16, kind="ExternalInput")
    cl_W1s_in = nc.dram_tensor("cl_W1s", [128, 13, 128], bf16, kind="ExternalInput")
    cl_b1s_in = nc.dram_tensor("cl_b1s", [128, 1], f32, kind="ExternalInput")
    cl_W2s_in = nc.dram_tensor("cl_W2s", [128, 256], bf16, kind="ExternalInput")
    cl_b2_in = nc.dram_tensor("cl_b2", [128, 2], f32, kind="ExternalInput")
    fc1_Ws_in = nc.dram_tensor("fc1_Ws", [128, 5, 256], bf16, kind="ExternalInput")
    fc1_bs_in = nc.dram_tensor("fc1_bs", [128, 2], f32, kind="ExternalInput")
    fc2_Ws_in = nc.dram_tensor("fc2_Ws", [128, 2, 256], bf16, kind="ExternalInput")
    fc2_b_in = nc.dram_tensor("fc2_b", [128, 2], f32, kind="ExternalInput")
    out_W_in = nc.dram_tensor("out_W_p", [128, 3, 1], bf16, kind="ExternalInput")
    out_b_in = nc.dram_tensor("out_b", [1, 1], f32, kind="ExternalInput")
    idx_b_in = nc.dram_tensor("idx_b", [128, TOTCOL], i16, kind="ExternalInput")
    idx_c_in = nc.dram_tensor("idx_c", [128, TOTCOL], i16, kind="ExternalInput")
    ld_in = nc.dram_tensor("ld", [128, NCHUNK], f32, kind="ExternalInput")
    dinv_in = nc.dram_tensor("dinv", [128, NGRP], f32, kind="ExternalInput")
    adc_in = nc.dram_tensor("adc", [128, NCHUNK, H], bf16, kind="ExternalInput")
    iota_in = nc.dram_tensor("iota", [128, 128], bf16, kind="ExternalInput")
    ident_in = nc.dram_tensor("ident", [128, 128], bf16, kind="ExternalInput")

    out_t = nc.dram_tensor("out", [B, 1], f32, kind="ExternalOutput")

    # --- internal DRAM ---
    T2_loc_q = [nc.dram_tensor(f"T2_loc_q{q}", [QROWS, T2W], bf16) for q in range(NQ)]
    T2_full = nc.dram_tensor("T2_full", [N, T2W], bf16)
    x2T_dram = nc.dram_tensor("x2T_dram", [7 * 128, NSH], bf16)
    Hg_loc = nc.dram_tensor("Hg_loc", [128, GPC], bf16)
    H_full = nc.dram_tensor("H_full", [128 * NCORES, GPC], bf16, addr_space="Shared")
    xcl_part = nc.dram_tensor("xcl_part", [128, 2, B], f32)
    xcl_sum = nc.dram_tensor("xcl_sum", [128, 2, B], f32)
    z4_part = nc.dram_tensor("z4_part", [128, 2, B], f32)
    z4_sum = nc.dram_tensor("z4_sum", [128, 2, B], f32)

    groups_b1 = [[0, 1, 2, 3], [4, 5, 6, 7]]
    groups_all = [list(range(NCORES))]

    with tile.TileContext(nc) as tc:
        nc.gpsimd.load_library(library_config.mlp)
        with tc.tile_pool(name="const", bufs=1) as cpool:
            iota = cpool.tile([128, 128], bf16)
            nc.sync.dma_start(iota[:], iota_in[:])
            ident = cpool.tile([128, 128], bf16)
            nc.sync.dma_start(ident[:], ident_in[:])
            idx_b = cpool.tile([128, TOTCOL], i16)
            nc.sync.dma_start(idx_b[:], idx_b_in[:])
            idx_c = cpool.tile([128, TOTCOL], i16)
            nc.sync.dma_start(idx_c[:], idx_c_in[:])
            ldc = cpool.tile([128, NCHUNK], f32)
            nc.sync.dma_start(ldc[:], ld_in[:])
            dinv = cpool.tile([128, NGRP], f32)
            nc.sync.dma_start(dinv[:], dinv_in[:])
            adc = cpool.tile([128, NCHUNK, H], bf16)
            nc.sync.dma_start(adc[:, :, :], adc_in[:, :, :])
            Wp77 = cpool.tile([128, H, F], bf16)
            nc.sync.dma_start(Wp77[:, :, :], Wp77_in[:, :, :])
            gat_bh = cpool.tile([128, H], f32)
            nc.sync.dma_start(gat_bh[:], gat_bh_in[:])
            gcn_brow = cpool.tile([1, HF], bf16)
            nc.sync.dma_start(gcn_brow[:], gcn_brow_in[:])
            ones1 = cpool.tile([1, 128], bf16)
            nc.vector.memset(ones1[:], 1.0)
            xcT = cpool.tile([128, 13, B], bf16)
            nc.sync.dma_start(xcT[:, :, :], xcT_in[:, :, :])

            # -------- cell-line MLP, feature-sharded; partial sum AllReduced --------
            # (runs early on the tensor engine; the AllReduce overlaps phase B)
            with tc.tile_pool(name="phCL", bufs=1) as clp, \
                 tc.tile_pool(name="psCL", bufs=1, space="PSUM") as psCL:
                cl_W1s = clp.tile([128, 13, 128], bf16)
                nc.sync.dma_start(cl_W1s[:, :, :], cl_W1s_in[:, :, :])
                cl_b1s = clp.tile([128, 1], f32)
                nc.sync.dma_start(cl_b1s[:], cl_b1s_in[:])
                cl_W2s = clp.tile([128, 256], bf16)
                nc.sync.dma_start(cl_W2s[:], cl_W2s_in[:])
                psz = psCL.tile([128, B], f32, tag="z")
                for kt in range(13):
                    nc.tensor.matmul(psz[:], cl_W1s[:, kt, :], xcT[:, kt, :],
                                     start=(kt == 0), stop=(kt == 12))
                zc1 = clp.tile([128, B], bf16)
                nc.scalar.activation(zc1[:], psz[:], Act.Relu, bias=cl_b1s[:, 0:1])
                xclp = clp.tile([128, 2, B], f32)
                for mt in range(2):
                    psz2 = psCL.tile([128, B], f32, tag="z2")
                    nc.tensor.matmul(psz2[:], cl_W2s[:, 128 * mt:128 * (mt + 1)],
                                     zc1[:], start=True, stop=True)
                    nc.scalar.activation(xclp[:, mt, :], psz2[:], Act.Copy)
                nc.sync.dma_start(xcl_part[:, :, :], xclp[:, :, :])
            nc.gpsimd.collective_compute(
                "AllReduce", mybir.AluOpType.add, replica_groups=groups_all,
                ins=[xcl_part[:, :, :]], outs=[xcl_sum[:, :, :]])

            # ---------------- Phase B: GAT conv (factorized) ----------------
            # Per 128-dst group: gather + alpha + weighted scatter-add. Per 4-group
            # super-group: one flipped per-head W matmul (x1T = W_h.T @ aggT, N=512),
            # then transpose back per group, ELU, scale by dinv, write T2 rows.
            with tc.tile_pool(name="phB", bufs=3) as bpool, \
                 tc.tile_pool(name="phBsk", bufs=3) as skpool, \
                 tc.tile_pool(name="phBs", bufs=3) as spool, \
                 tc.tile_pool(name="phBt", bufs=2) as tpool, \
                 tc.tile_pool(name="psB", bufs=1, space="PSUM") as psB, \
                 tc.tile_pool(name="psBt", bufs=1, space="PSUM") as psT, \
                 tc.tile_pool(name="psBx", bufs=3, space="PSUM") as psX:
                kbase = 0
                for sg in range(NGRP // 4):
                    aggnT_sg = tpool.tile([128, H, 512], bf16, tag="aT")
                    for gi in range(4):
                        g = 4 * sg + gi
                        Cg = C[g]
                        cc = group_cols[g]
                        G = bpool.tile([128, Cg, T1W], bf16, tag="G")
                        nc.gpsimd.dma_gather(
                            G[:, :, :], T1_in[:, :], idx_b[:, cc:cc + 8 * Cg],
                            128 * Cg, 128 * Cg, T1W)
                        # alpha -> e for all chunks of the group at once
                        al = spool.tile([128, Cg, H], f32, tag="al")
                        asl = adc[:, kbase:kbase + Cg, :]
                        nc.vector.scalar_tensor_tensor(al[:, :, :], asl, 0.2,
                                                       asl, AluOp.mult, AluOp.max)
                        if need_clamp:
                            nc.vector.tensor_scalar(al[:, :, :], al[:, :, :], 60.0,
                                                    None, AluOp.min)
                        ee = spool.tile([128, Cg, H], bf16, tag="ee")
                        nc.scalar.activation(ee[:, :, :], al[:, :, :], Act.Exp)
                        # select matrices for all chunks in one op
                        Sk_all = skpool.tile([128, Cg, 128], bf16, tag="Sk")
                        nc.vector.tensor_tensor(
                            Sk_all[:, :, :],
                            ldc[:, kbase:kbase + Cg].rearrange("p (c o) -> p c o", o=1)
                                .broadcast_to((128, Cg, 128)),
                            iota[:, :].rearrange("p (o j) -> p o j", o=1)
                                .broadcast_to((128, Cg, 128)),
                            AluOp.is_equal)
                        # weighted messages [e*x | e] per head (78 cols), one 4D op
                        Gw = spool.tile([128, Cg, HF + H], bf16, tag="Gw")
                        e4 = ee[:, :, :].rearrange("p c (h o) -> p c h o", o=1) \
                            .broadcast_to((128, Cg, H, F + 1))
                        x4 = G[:, :, 0:78].rearrange("p c (o f) -> p c o f", o=1) \
                            .broadcast_to((128, Cg, H, F + 1))
                        nc.vector.tensor_tensor(
                            Gw[:, :, :].rearrange("p c (h f) -> p c h f", f=F + 1),
                            e4, x4, AluOp.mult)
                        pg = psB.tile([128, HF + H], f32, tag="agg")
                        for k in range(Cg):
                            nc.tensor.matmul(pg[:, 0:512], Sk_all[:, k, :],
                                             Gw[:, k, 0:512],
                                             start=(k == 0), stop=(k == Cg - 1))
                            nc.tensor.matmul(pg[:, 512:HF + H], Sk_all[:, k, :],
                                             Gw[:, k, 512:HF + H],
                                             start=(k == 0), stop=(k == Cg - 1))
                        kbase += Cg
                        # normalize by the denominator, transpose into the super-group
                        pgs = spool.tile([128, HF + H], f32, tag="pgs")
                        nc.scalar.activation(pgs[:], pg[:], Act.Copy)
                        pg4 = pgs[:].rearrange("p (h f) -> p h f", f=F + 1)
                        rden = spool.tile([128, H], f32, tag="rden")
                        nc.vector.reciprocal(rden[:], pg4[:, :, F:F + 1]
                                             .rearrange("p h o -> p (h o)"))
                        aggn = spool.tile([128, HF], bf16, tag="aggn")
                        r3 = rden[:].rearrange("p (h o) -> p h o", o=1) \
                            .broadcast_to((128, H, F))
                        nc.vector.tensor_tensor(
                            aggn[:].rearrange("p (h f) -> p h f", f=F),
                            pg4[:, :, 0:F], r3, AluOp.mult)
                        ptb = psT.tile([128, H, 128], bf16, tag="tr")
                        for h in range(H):
                            nc.tensor.transpose(ptb[0:77, h, :],
                                                aggn[:, F * h:F * (h + 1)], ident[:])
                        nc.scalar.activation(aggnT_sg[:, :, 128 * gi:128 * (gi + 1)],
                                             ptb[:, :, :], Act.Copy)
                    # per-head W apply over all 512 dst of the super-group
                    x1sb = tpool.tile([128, H, 512], bf16, tag="x1sb")
                    for h in range(H):
                        psx = psX.tile([128, 512], f32, tag="x1")
                        nc.tensor.matmul(psx[0:77, :], Wp77[0:77, h, 0:77],
                                         aggnT_sg[0:77, h, :], start=True, stop=True)
                        nc.scalar.activation(x1sb[0:77, h, :], psx[0:77, :],
                                             Act.Identity,
                                             bias=gat_bh[0:77, h:h + 1])
                    # back to row-major per group: transpose, ELU, *dinv, T2 write
                    for gi in range(4):
                        g = 4 * sg + gi
                        ptc = psT.tile([128, H, F + 1], bf16, tag="ptc")
                        for h in range(H):
                            nc.tensor.transpose(
                                ptc[:, h, 0:F],
                                x1sb[0:77, h, 128 * gi:128 * (gi + 1)],
                                ident[0:77, 0:77])
                        zr = spool.tile([128, H, F + 1], bf16, tag="zr")
                        nc.scalar.activation(zr[:, :, :], ptc[:, :, :], Act.Copy)
                        m = spool.tile([128, H * (F + 1)], bf16, tag="m")
                        nc.vector.tensor_scalar(m[:], zr[:, :, :]
                                                .rearrange("p h f -> p (h f)"),
                                                0.0, None, AluOp.min)
                        e2 = spool.tile([128, H * (F + 1)], bf16, tag="e2")
                        nc.scalar.activation(e2[:], m[:], Act.Exp)
                        x1f = spool.tile([128, H, F + 1], f32, tag="x1f")
                        nc.vector.scalar_tensor_tensor(
                            x1f[:, :, :].rearrange("p h f -> p (h f)"), e2[:], -1.0,
                            zr[:, :, :].rearrange("p h f -> p (h f)"),
                            AluOp.add, AluOp.max)
                        T2row = bpool.tile([128, T2W], bf16, tag="T2row")
                        nc.scalar.activation(T2row[:, 0:HF]
                                             .rearrange("p (h f) -> p h f", f=F),
                                             x1f[:, :, 0:F], Act.Copy,
                                             scale=dinv[:, g:g + 1])
                        q, j = g // QGRP, g % QGRP
                        nc.sync.dma_start(T2_loc_q[q][128 * j:128 * (j + 1), 0:HF],
                                          T2row[:, 0:HF])
                        if j == QGRP - 1:
                            nc.gpsimd.collective_compute(
                                "AllGather", mybir.AluOpType.bypass,
                                replica_groups=groups_b1,
                                ins=[T2_loc_q[q][:, :]],
                                outs=[T2_full[R * QROWS * q:R * QROWS * (q + 1), :]])

            gcn_W = cpool.tile([128, 7, HF], bf16)
            nc.sync.dma_start(gcn_W[:, :, :], gcn_W_in[:, :, :])

            # ---------------- Phase C: GCN conv ----------------
            # Aggregate per 128-dst group; batch 4 groups into a 512-node super-group and
            # produce x2T directly: x2T[fo, n] = relu(sum_fi W[fi, fo] * aggT[fi, n] + b).
            with tc.tile_pool(name="phC", bufs=2) as cpool2, \
                 tc.tile_pool(name="phCsk", bufs=2) as skpool2, \
                 tc.tile_pool(name="phCs", bufs=2) as cspool, \
                 tc.tile_pool(name="phCt", bufs=2) as tpool2, \
                 tc.tile_pool(name="psC", bufs=2, space="PSUM") as psC, \
                 tc.tile_pool(name="psCt", bufs=2, space="PSUM") as psCt, \
                 tc.tile_pool(name="psCx", bufs=2, space="PSUM") as psCx:
                onesd = cpool.tile([1, 512], bf16)
                nc.vector.memset(onesd[:], 1.0)
                kbase = 0
                for sg in range(NGRP // 4):
                    aggT_sg = tpool2.tile([128, 7, 512], bf16, tag="aggT")
                    for gi in range(4):
                        g = 4 * sg + gi
                        Cg = C[g]
                        cc = group_cols[g]
                        G2 = cpool2.tile([128, Cg, T2W], bf16, tag="G2")
                        nc.gpsimd.dma_gather(
                            G2[:, :, :], T2_full[:, :], idx_c[:, cc:cc + 8 * Cg],
                            128 * Cg, 128 * Cg, T2W)
                        Sk_all = skpool2.tile([128, Cg, 128], bf16, tag="S2")
                        nc.vector.tensor_tensor(
                            Sk_all[:, :, :],
                            ldc[:, kbase:kbase + Cg].rearrange("p (c o) -> p c o", o=1)
                                .broadcast_to((128, Cg, 128)),
                            iota[:, :].rearrange("p (o j) -> p o j", o=1)
                                .broadcast_to((128, Cg, 128)),
                            AluOp.is_equal)
                        pg = psC.tile([128, HF], f32, tag="acc2")
                        for k in range(Cg):
                            nc.tensor.matmul(pg[:, 0:512], Sk_all[:, k, :],
                                             G2[:, k, 0:512],
                                             start=(k == 0), stop=(k == Cg - 1))
                            nc.tensor.matmul(pg[:, 512:HF], Sk_all[:, k, :],
                                             G2[:, k, 512:HF],
                                             start=(k == 0), stop=(k == Cg - 1))
                            kbase += 1
                        agg = cspool.tile([128, HF], bf16, tag="agg")
                        nc.scalar.activation(agg[:], pg[:], Act.Copy,
                                             scale=dinv[:, g:g + 1])
                        ptb = psCt.tile([128, 7, 128], bf16, tag="tr")
                        for ft in range(7):
                            kf = min(128, HF - 128 * ft)
                            nc.tensor.transpose(ptb[0:kf, ft, :],
                                                agg[:, 128 * ft:128 * ft + kf], ident[:])
                        nc.scalar.activation(aggT_sg[:, :, 128 * gi:128 * (gi + 1)],
                                             ptb[:, :, :], Act.Copy)
                    x2T_sb = cspool.tile([128, 7, 512], bf16, tag="x2T")
                    for mt in range(7):
                        mm = min(128, HF - 128 * mt)
                        psx = psCx.tile([128, 512], f32, tag="x2ps")
                        for kt in range(7):
                            kf = min(128, HF - 128 * kt)
                            nc.tensor.matmul(psx[0:mm, :],
                                             gcn_W[0:kf, kt, 128 * mt:128 * mt + mm],
                                             aggT_sg[0:kf, kt, :],
                                             start=(kt == 0), stop=False)
                        nc.tensor.matmul(psx[0:mm, :],
                                         gcn_brow[0:1, 128 * mt:128 * mt + mm],
                                         onesd[0:1, :], start=False, stop=True)
                        nc.scalar.activation(x2T_sb[0:mm, mt, :], psx[0:mm, :], Act.Relu)
                        nc.sync.dma_start(
                            x2T_dram[128 * mt:128 * mt + mm, 512 * sg:512 * (sg + 1)],
                            x2T_sb[0:mm, mt, :])

            # ---------------- Phase D: pooling ----------------
            with tc.tile_pool(name="pooled", bufs=1) as plpool:
                pooled = plpool.tile([128, 14, 128], bf16)
                nc.vector.memset(pooled[:, 6, :], 0.0)
                nc.vector.memset(pooled[:, 13, :], 0.0)
                with tc.tile_pool(name="phD", bufs=2) as dpool:
                    for ft in range(7):
                        kf = min(128, HF - 128 * ft)
                        strip = dpool.tile([128, NSH], bf16, tag="strip")
                        nc.sync.dma_start(strip[0:kf, :],
                                          x2T_dram[128 * ft:128 * ft + kf, :])
                        seg = strip[0:kf, :].rearrange("p (gr n) -> p gr n", n=NPG)
                        nc.vector.tensor_reduce(pooled[0:kf, ft, :], seg,
                                                mybir.AxisListType.X, AluOp.max)
                        sm = dpool.tile([128, 128], f32, tag="sm")
                        nc.vector.tensor_reduce(sm[0:kf, :], seg,
                                                mybir.AxisListType.X, AluOp.add)
                        nc.vector.tensor_scalar(pooled[0:kf, 7 + ft, :], sm[0:kf, :],
                                                1.0 / NPG, None, AluOp.mult)

                # ---------------- Phase E: fcg1 / fcg2 ----------------
                with tc.tile_pool(name="phE", bufs=1) as epool, \
                     tc.tile_pool(name="psE", bufs=2, space="PSUM") as psE:
                    fcg1_W = epool.tile([128, 14, 1536], bf16)
                    nc.sync.dma_start(fcg1_W[:, :, :], fcg1_W_in[:, :, :])
                    fcg1_b = epool.tile([128, 12], f32)
                    nc.sync.dma_start(fcg1_b[:], fcg1_b_in[:])
                    fcg2_W = epool.tile([128, 12, 128], bf16)
                    nc.sync.dma_start(fcg2_W[:, :, :], fcg2_W_in[:, :, :])
                    fcg2_b = epool.tile([128, 1], f32)
                    nc.sync.dma_start(fcg2_b[:], fcg2_b_in[:])
                    z1T = epool.tile([128, 12, 128], bf16)
                    for mt in range(12):
                        psz = psE.tile([128, 128], f32, tag="ze")
                        for kt in range(14):
                            nc.tensor.matmul(psz[:], fcg1_W[:, kt, 128 * mt:128 * (mt + 1)],
                                             pooled[:, kt, :], start=(kt == 0), stop=(kt == 13))
                        nc.scalar.activation(z1T[:, mt, :], psz[:], Act.Relu,
                                             bias=fcg1_b[:, mt:mt + 1])
                    psh = psE.tile([128, GPC], f32, tag="he")
                    for kt in range(12):
                        nc.tensor.matmul(psh[:], fcg2_W[:, kt, :], z1T[:, kt, :],
                                         start=(kt == 0), stop=(kt == 11))
                    hdT = epool.tile([128, GPC], bf16)
                    nc.scalar.activation(hdT[:], psh[:], Act.Relu, bias=fcg2_b[:, 0:1])
                    nc.sync.dma_start(Hg_loc[:, :], hdT[:])

            nc.gpsimd.collective_compute(
                "AllGather", mybir.AluOpType.bypass, replica_groups=groups_all,
                ins=[Hg_loc[:, :]], outs=[H_full[:, :]])

            # ---------------- Phase F: fusion MLP (feature-sharded) ----------------
            with tc.tile_pool(name="phFa", bufs=1) as fpool, \
                 tc.tile_pool(name="psF", bufs=2, space="PSUM") as psF:
                cl_b2 = fpool.tile([128, 2], f32)
                nc.sync.dma_start(cl_b2[:], cl_b2_in[:])
                xcls = fpool.tile([128, 2, B], f32)
                nc.sync.dma_start(xcls[:, :, :], xcl_sum[:, :, :])
                xclT = fpool.tile([128, 2, B], bf16)
                for mt in range(2):
                    nc.scalar.activation(xclT[:, mt, :], xcls[:, mt, :], Act.Relu,
                                         bias=cl_b2[:, mt:mt + 1])
                h1T = fpool.tile([128, B], bf16)
                h2T = fpool.tile([128, B], bf16)
                for r in range(R):
                    nc.sync.dma_start(h1T[:, GPC * r:GPC * (r + 1)],
                                      H_full[128 * r:128 * (r + 1), :])
                    nc.sync.dma_start(h2T[:, GPC * r:GPC * (r + 1)],
                                      H_full[128 * (R + r):128 * (R + r + 1), :])
                xtcT = xcT[0:10, 12, :]   # xtc rows live at packed rows 1536:1546
                fc1_Ws = fpool.tile([128, 5, 256], bf16)
                nc.sync.dma_start(fc1_Ws[:, :, :], fc1_Ws_in[:, :, :])
                fc1_bs = fpool.tile([128, 2], f32)
                nc.sync.dma_start(fc1_bs[:], fc1_bs_in[:])
                rhs1 = [h1T[:, :], h2T[:, :], xclT[:, 0, :], xclT[:, 1, :], xtcT]
                z3T = fpool.tile([128, 2, B], bf16)
                for mt in range(2):
                    psz = psF.tile([128, B], f32, tag="zf")
                    for kt in range(5):
                        kf = 10 if kt == 4 else 128
                        nc.tensor.matmul(psz[:], fc1_Ws[0:kf, kt, 128 * mt:128 * (mt + 1)],
                                         rhs1[kt], start=(kt == 0), stop=(kt == 4))
                    nc.scalar.activation(z3T[:, mt, :], psz[:], Act.Relu,
                                         bias=fc1_bs[:, mt:mt + 1])
                fc2_Ws = fpool.tile([128, 2, 256], bf16)
                nc.sync.dma_start(fc2_Ws[:, :, :], fc2_Ws_in[:, :, :])
                z4p = fpool.tile([128, 2, B], f32)
                for mt in range(2):
                    psz = psF.tile([128, B], f32, tag="zf")
                    for kt in range(2):
                        nc.tensor.matmul(psz[:], fc2_Ws[:, kt, 128 * mt:128 * (mt + 1)],
                                         z3T[:, kt, :], start=(kt == 0), stop=(kt == 1))
                    nc.scalar.activation(z4p[:, mt, :], psz[:], Act.Copy)
                nc.sync.dma_start(z4_part[:, :, :], z4p[:, :, :])
            nc.gpsimd.collective_compute(
                "AllReduce", mybir.AluOpType.add, replica_groups=groups_all,
                ins=[z4_part[:, :, :]], outs=[z4_sum[:, :, :]])
            with tc.tile_pool(name="phFo", bufs=1) as fopool, \
                 tc.tile_pool(name="psFo", bufs=1, space="PSUM") as psFo:
                fc2_b = fopool.tile([128, 2], f32)
                nc.sync.dma_start(fc2_b[:], fc2_b_in[:])
                z4s = fopool.tile([128, 2, B], f32)
                nc.sync.dma_start(z4s[:, :, :], z4_sum[:, :, :])
                z4T = fopool.tile([128, 2, B], bf16)
                for mt in range(2):
                    nc.scalar.activation(z4T[:, mt, :], z4s[:, mt, :], Act.Relu,
                                         bias=fc2_b[:, mt:mt + 1])
                out_W = fopool.tile([128, 3, 1], bf16)
                nc.sync.dma_start(out_W[:, :, :], out_W_in[:, :, :])
                out_b = fopool.tile([1, 1], f32)
                nc.sync.dma_start(out_b[:], out_b_in[:])
                pso = psFo.tile([1, B], f32, tag="po")
                rhs_o = [z4T[:, 0, :], z4T[:, 1, :], xtcT]
                for kt in range(3):
                    kf = 10 if kt == 2 else 128
                    nc.tensor.matmul(pso[:], out_W[0:kf, kt, :], rhs_o[kt],
                                     start=(kt == 0), stop=(kt == 2))
                fin = fopool.tile([1, B], f32)
                nc.vector.scalar_tensor_tensor(fin[:], pso[:], 1.0, out_b[:]
                                               .broadcast_to((1, B)),
                                               AluOp.mult, AluOp.add)
                nc.vector.tensor_scalar(fin[:], fin[:], 100.0, -100.0,
                                        AluOp.min, AluOp.max)
                nc.sync.dma_start(out_t[:, :].rearrange("b o -> o b"), fin[:])

    nc.compile()
    return nc


def kernel(**inputs):
    in_maps, struct = _build_host_inputs(inputs)
    nc = _build_program(struct)
    import os
    trace = bool(int(os.environ.get("GNN_TRACE", "0")))
    res = run_bass_kernel_spmd(nc, in_maps, core_ids=list(range(NCORES)), trace=trace)
    kernel.last_result = res
    return np.asarray(res.results[0]["out"]).reshape(B, 1).astype(np.float32)
